# revision 1
# baseline (speedup 1.0000x reference)
"""Segment-softmax GNN attention kernel for 8 Trainium2 NeuronCores.

Math (reference): latent = leaky_relu(x @ W + b, 0.2)  -> [E, 1]
                  out = scatter_softmax(latent, index) -> [E, 1]

Strategy (regular access patterns only — no indirect DMA):
  Host: stable-sort edges by destination segment; shard segment-aligned
  across 8 cores (6250 segments each, so no cross-core reduction); pad
  every segment to a multiple of G=4 edges (dummy x-rows chosen so their
  logit is -1e33 -> exp 0).
  Device per core, all static APs:
    A) stream x tiles; DVE matvec vs replicated W; leaky-relu =
       0.2*z' + 0.8*relu(z') (z' = z+b); exp; keep e=exp in SBUF and
       reduce per 4-edge block -> block sums B4 (segment boundaries are
       block-aligned by construction).
    B) reload B4 with a +-11-block halo; per-block segment denominator
       D4[q] = sum_v B4[q+v-11] * V[q,v] where V is a host-built banded
       0/1 membership (segments span <= 12 blocks); reciprocal -> R4.
    C) out[e] = e[e] * R4[block(e)]; store in padded-edge order.
  Host: drop padding, inverse-permute.
  No max-subtraction needed: logits ~ N(0,1) so exp is safe in f32.
"""

import os
import sys

sys.path.insert(0, "/opt/trn_rl_repo")

import numpy as np

N_NODES = 50000
N_CORES = 8
SEG_PER_CORE = N_NODES // N_CORES          # 6250
D = 128
EDGE_TILE = 2048                           # edges per phase-A tile
CPP = EDGE_TILE // 128                     # 16 edges per partition per tile
G = 4                                      # block granularity (segment pad)
NEG_SLOPE = 0.2
VW = 23                                    # membership band width (+-11 blocks)
GUARD = 32                                 # zeroed guard blocks each side of B4

_compiled_cache = {}


def _build_graph(E_pad: int):
    import concourse.bacc as bacc
    import concourse.tile as tile
    from concourse import bass, mybir

    f32 = mybir.dt.float32
    n_xt = E_pad // EDGE_TILE
    NB = E_pad // G                        # blocks per core
    BCOL = NB // 128                       # block columns per partition
    BHW = BCOL + VW - 1                    # halo'd width
    ECOL = E_pad // 128                    # e4_sb columns (= n_xt * CPP)
    CB = CPP // G                          # blocks per partition per tile (4)

    nc = bacc.Bacc("TRN2", target_bir_lowering=False, debug=False,
                   num_devices=N_CORES)

    xs_d = nc.dram_tensor("xs", [E_pad, D], f32, kind="ExternalInput")
    w_d = nc.dram_tensor("wrep", [1, EDGE_TILE], f32, kind="ExternalInput")
    b_d = nc.dram_tensor("bvec", [1, 1], f32, kind="ExternalInput")
    b02_d = nc.dram_tensor("b02", [1, 1], f32, kind="ExternalInput")
    v_d = nc.dram_tensor("vmem", [128, VW, BCOL], f32, kind="ExternalInput")
    out_d = nc.dram_tensor("out", [E_pad, 1], f32, kind="ExternalOutput")
    b4_d = nc.dram_tensor("b4", [NB + 2 * GUARD, 1], f32)
    r4_d = nc.dram_tensor("r4", [NB, 1], f32)

    AP = bass.AP
    ALU = mybir.AluOpType
    ACT = mybir.ActivationFunctionType

    with tile.TileContext(nc) as tc:
        with (
            tc.tile_pool(name="consts", bufs=1) as consts,
            tc.tile_pool(name="xin", bufs=3) as xin,
            tc.tile_pool(name="prod", bufs=2) as prod,
            tc.tile_pool(name="small", bufs=6) as small,
            tc.tile_pool(name="keep", bufs=1) as keep,
            tc.tile_pool(name="bwork", bufs=3) as bwork,
            tc.tile_pool(name="cio", bufs=4) as cio,
        ):
            # --- constants ---
            wb = consts.tile([128, CPP, D], f32)
            nc.sync.dma_start(
                out=wb[:],
                in_=AP(tensor=w_d, offset=0, ap=[[0, 128], [D, CPP], [1, D]]),
            )
            bb = consts.tile([128, 1], f32)
            nc.sync.dma_start(
                out=bb[:], in_=AP(tensor=b_d, offset=0, ap=[[0, 128], [1, 1]])
            )
            bb02 = consts.tile([128, 1], f32)
            nc.sync.dma_start(
                out=bb02[:], in_=AP(tensor=b02_d, offset=0, ap=[[0, 128], [1, 1]])
            )
            zg = consts.tile([1, GUARD], f32)
            nc.vector.memset(zg[:], 0.0)
            nc.sync.dma_start(out=b4_d[0:GUARD, :], in_=zg[:])
            nc.sync.dma_start(out=b4_d[GUARD + NB:GUARD + NB + GUARD, :], in_=zg[:])

            e4_sb = keep.tile([128, ECOL], f32)     # all exp values, SBUF-resident
            vmem = keep.tile([128, VW, BCOL], f32)
            nc.sync.dma_start(out=vmem[:], in_=v_d[:, :, :])

            # --- phase A: logits -> exp -> block sums ---
            for i in range(n_xt):
                xt = xin.tile([128, CPP, D], f32)
                nc.sync.dma_start(
                    out=xt[:],
                    in_=AP(tensor=xs_d, offset=i * EDGE_TILE * D,
                           ap=[[CPP * D, 128], [D, CPP], [1, D]]),
                )
                pt = prod.tile([128, CPP, D], f32)
                nc.vector.tensor_tensor(out=pt[:], in0=xt[:], in1=wb[:],
                                        op=ALU.mult)
                zt = small.tile([128, CPP], f32)
                nc.vector.tensor_reduce(out=zt[:], in_=pt[:],
                                        axis=mybir.AxisListType.X, op=ALU.add)
                # leaky_relu(z+b) = 0.2*z + 0.2*b + 0.8*relu(z+b)
                rt = small.tile([128, CPP], f32)
                nc.scalar.activation(out=rt[:], in_=zt[:], func=ACT.Relu,
                                     bias=bb[:, 0:1], scale=1.0)
                t1 = small.tile([128, CPP], f32)
                nc.vector.tensor_scalar(out=t1[:], in0=zt[:], scalar1=NEG_SLOPE,
                                        scalar2=bb02[:, 0:1], op0=ALU.mult,
                                        op1=ALU.add)
                t2 = small.tile([128, CPP], f32)
                nc.vector.tensor_scalar(out=t2[:], in0=rt[:],
                                        scalar1=1.0 - NEG_SLOPE, scalar2=None,
                                        op0=ALU.mult)
                lt = small.tile([128, CPP], f32)
                nc.vector.tensor_tensor(out=lt[:], in0=t1[:], in1=t2[:],
                                        op=ALU.add)
                et = e4_sb[:, i * CPP:(i + 1) * CPP]
                nc.scalar.activation(out=et, in_=lt[:], func=ACT.Exp)
                b4t = small.tile([128, CB], f32)
                nc.vector.tensor_reduce(
                    out=b4t[:], in_=et.rearrange("p (cb g) -> p cb g", g=G),
                    axis=mybir.AxisListType.X, op=ALU.add)
                nc.scalar.dma_start(
                    out=AP(tensor=b4_d, offset=GUARD + i * (EDGE_TILE // G),
                           ap=[[CB, 128], [1, CB]]),
                    in_=b4t[:],
                )

            # --- phase B: banded membership -> per-block denominators ---
            b4h = bwork.tile([128, BHW], f32)
            nc.sync.dma_start(
                out=b4h[:],
                in_=AP(tensor=b4_d, offset=GUARD - (VW // 2),
                       ap=[[BCOL, 128], [1, BHW]]),
            )
            d4 = bwork.tile([128, BCOL], f32)
            nc.vector.tensor_tensor(out=d4[:], in0=b4h[:, 0:BCOL],
                                    in1=vmem[:, 0, :], op=ALU.mult)
            for v in range(1, VW):
                tmp = bwork.tile([128, BCOL], f32, tag="btmp")
                nc.vector.tensor_tensor(out=tmp[:], in0=b4h[:, v:v + BCOL],
                                        in1=vmem[:, v, :], op=ALU.mult)
                nc.vector.tensor_tensor(out=d4[:], in0=d4[:], in1=tmp[:],
                                        op=ALU.add)
            d4e = bwork.tile([128, BCOL], f32)
            nc.vector.tensor_scalar(out=d4e[:], in0=d4[:], scalar1=1e-12,
                                    scalar2=None, op0=ALU.add)
            r4 = bwork.tile([128, BCOL], f32)
            nc.vector.reciprocal(out=r4[:], in_=d4e[:])
            nc.sync.dma_start(
                out=AP(tensor=r4_d, offset=0, ap=[[BCOL, 128], [1, BCOL]]),
                in_=r4[:],
            )

            # --- phase C: out = e * R4[block] ---
            for i in range(n_xt):
                r4t = cio.tile([128, CB], f32)
                nc.sync.dma_start(
                    out=r4t[:],
                    in_=AP(tensor=r4_d, offset=i * (EDGE_TILE // G),
                           ap=[[CB, 128], [1, CB]]),
                )
                ot = cio.tile([128, CPP], f32)
                r4t_ap = r4t[:]
                r4b = AP(tensor=r4t_ap.tensor, offset=r4t_ap.offset,
                         ap=[r4t_ap.ap[0], r4t_ap.ap[1], [0, G]])
                nc.vector.tensor_tensor(
                    out=ot[:].rearrange("p (cb g) -> p cb g", g=G),
                    in0=e4_sb[:, i * CPP:(i + 1) * CPP].rearrange(
                        "p (cb g) -> p cb g", g=G),
                    in1=r4b, op=ALU.mult)
                nc.scalar.dma_start(
                    out=AP(tensor=out_d, offset=i * EDGE_TILE,
                           ap=[[CPP, 128], [1, CPP]]),
                    in_=ot[:],
                )

    nc.compile()
    return nc


def _host_prep(x, W, b, index):
    """Sort/pad/shard on host; returns per-core in_maps plus reassembly info."""
    x = np.ascontiguousarray(np.asarray(x, dtype=np.float32))
    W = np.asarray(W, dtype=np.float32).reshape(D)
    b = np.asarray(b, dtype=np.float32).reshape(1)
    idx = np.asarray(index).astype(np.int64).ravel()
    E = idx.shape[0]

    order = np.argsort(idx, kind="stable")
    idx_s = idx[order]
    counts = np.bincount(idx_s, minlength=N_NODES).astype(np.int64)
    seg_starts = np.zeros(N_NODES + 1, dtype=np.int64)
    np.cumsum(counts, out=seg_starts[1:])
    plen = ((counts + G - 1) // G) * G                     # padded lengths

    core_e = seg_starts[np.arange(N_CORES + 1) * SEG_PER_CORE]
    pcum = np.zeros(N_NODES + 1, dtype=np.int64)
    np.cumsum(plen, out=pcum[1:])
    core_p = pcum[np.arange(N_CORES + 1) * SEG_PER_CORE]   # padded core bounds
    pcounts = np.diff(core_p)
    E_pad = int(np.ceil(max(pcounts.max(), 1) / EDGE_TILE) * EDGE_TILE)

    x_sorted = x[order]
    wrep = np.tile(W, CPP).reshape(1, EDGE_TILE)
    bvec = b.reshape(1, 1).astype(np.float32)
    b02 = (NEG_SLOPE * b).reshape(1, 1).astype(np.float32)
    wsq = float(W @ W)
    dummy_row = (-1e33 / max(wsq, 1e-30)) * W              # logit -> -1e33

    NB = E_pad // G
    BCOL = NB // 128
    HB = VW // 2

    in_maps = []
    reasm = []
    for k in range(N_CORES):
        e0, e1 = int(core_e[k]), int(core_e[k + 1])
        cnt = e1 - e0
        s0 = k * SEG_PER_CORE
        scnt = counts[s0:s0 + SEG_PER_CORE]
        sstart = seg_starts[s0:s0 + SEG_PER_CORE] - e0     # compact local starts
        pstart = pcum[s0:s0 + SEG_PER_CORE] - int(core_p[k])  # padded local starts

        seg_local = (idx_s[e0:e1] - s0).astype(np.int64)
        pos_in_seg = np.arange(cnt, dtype=np.int64) - sstart[seg_local]
        ppos = pstart[seg_local] + pos_in_seg              # padded slot per edge

        xs = np.broadcast_to(dummy_row, (E_pad, D)).copy()
        xs[ppos] = x_sorted[e0:e1]

        # block -> segment id (-1 for tail padding)
        nb = (plen[s0:s0 + SEG_PER_CORE] // G).astype(np.int64)
        bseg = np.full(NB, -1, dtype=np.int64)
        bseg[:int(nb.sum())] = np.repeat(np.arange(SEG_PER_CORE), nb)
        bpad = np.full(NB + 2 * HB, -2, dtype=np.int64)
        bpad[HB:HB + NB] = bseg
        V = np.empty((NB, VW), dtype=np.float32)
        for v in range(VW):
            V[:, v] = (bpad[v:v + NB] == bseg)
        vmem = np.ascontiguousarray(
            V.reshape(128, BCOL, VW).transpose(0, 2, 1)).astype(np.float32)

        in_maps.append({
            "xs": xs, "wrep": wrep, "bvec": bvec, "b02": b02, "vmem": vmem,
        })
        reasm.append(ppos)

    return in_maps, reasm, order, core_e, E_pad, E


def _emulate_core(m, E_pad):
    """Numpy emulation of the device graph for one core (host-logic check)."""
    xs, wrep, bvec, b02 = m["xs"], m["wrep"], m["bvec"], m["b02"]
    vmem = m["vmem"]
    NB = E_pad // G
    BCOL = NB // 128
    z = xs @ wrep.ravel()[:D]
    r = np.maximum(z + bvec.ravel()[0], 0.0)
    l = NEG_SLOPE * z + b02.ravel()[0] + (1.0 - NEG_SLOPE) * r
    e = np.exp(l).astype(np.float32)
    B4 = e.reshape(NB, G).sum(axis=1)
    HB = VW // 2
    B4p = np.concatenate([np.zeros(HB, np.float32), B4, np.zeros(HB, np.float32)])
    V = vmem.transpose(0, 2, 1).reshape(NB, VW)
    D4 = np.zeros(NB, np.float32)
    for v in range(VW):
        D4 += B4p[v:v + NB] * V[:, v]
    R4 = 1.0 / (D4 + 1e-12)
    return (e * np.repeat(R4, G)).astype(np.float32)


LAST_RESULTS = None  # BassKernelResults from the most recent run


def kernel(x, W, b, index):
    global LAST_RESULTS
    in_maps, reasm, order, core_e, E_pad, E = _host_prep(x, W, b, index)

    if os.environ.get("KERNEL_EMULATE"):
        outs = [_emulate_core(m, E_pad) for m in in_maps]
    else:
        from concourse.bass_utils import run_bass_kernel_spmd

        if E_pad not in _compiled_cache:
            _compiled_cache[E_pad] = _build_graph(E_pad)
        nc = _compiled_cache[E_pad]
        trace = bool(os.environ.get("BASS_TRACE"))
        LAST_RESULTS = run_bass_kernel_spmd(
            nc, in_maps, list(range(N_CORES)), trace=trace,
        )
        outs = [r["out"] for r in LAST_RESULTS.results]

    out_sorted = np.empty(E, dtype=np.float32)
    for k in range(N_CORES):
        e0, e1 = int(core_e[k]), int(core_e[k + 1])
        out_sorted[e0:e1] = np.asarray(outs[k]).ravel()[reasm[k]]
    out = np.empty(E, dtype=np.float32)
    out[order] = out_sorted
    return out[:, None]



# revision 3
# speedup vs baseline: 1.2218x; 1.2218x over previous
"""Segment-softmax GNN attention kernel for 8 Trainium2 NeuronCores.

Math (reference): latent = leaky_relu(x @ W + b, 0.2)  -> [E, 1]
                  out = scatter_softmax(latent, index) -> [E, 1]

Strategy (regular access patterns only — no indirect DMA):
  Host: stable-sort edges by destination segment; shard segment-aligned
  across 8 cores (6250 segments each -> no cross-core reduction); pad
  every segment to a multiple of G=4 edges (dummy rows whose logit is
  -40 -> exp underflows to 0 in f16); convert x to float16 (DMA halves;
  dot-product error ~0.1% vs the 2e-2 budget).
  Device layout: partition p owns the contiguous edge chunk
  [p*EPP, (p+1)*EPP) so per-4-edge block sums land directly in the
  banded-membership layout — no DRAM transpose roundtrip.
  Device per core, all static APs:
    A) stream x tiles [128, T, D] f16; DVE mult (2x mode) + DVE reduce
       over D (2x, f16 accum); leaky-relu smalls on GpSimd; Relu/Exp on
       Scalar; e=exp kept in SBUF (f16); per-4-edge block sums B4
       accumulate into an SBUF tile in band layout.
    B) halo-exchange B4 partition edges via tiny DRAM bounce; banded
       0/1 membership V (f16, host-built; segments span <= 12 blocks):
       D4[q] = sum_v B4[q+v-11]*V[q,v]; add eps; reciprocal -> R4 (f32).
    C) out[e] = e[e] * R4[block(e)] (f32); single big store.
  Host: drop padding, inverse-permute.
  No max-subtraction needed: logits ~ N(0,1) so exp is safe.
"""

import os
import sys

sys.path.insert(0, "/opt/trn_rl_repo")

import numpy as np

N_NODES = 50000
N_CORES = 8
SEG_PER_CORE = N_NODES // N_CORES          # 6250
D = 128
T = 32                                     # edges per partition per tile
G = 4                                      # block granularity (segment pad)
NEG_SLOPE = 0.2
VW = 23                                    # membership band width (+-11 blocks)
HB = VW // 2
DUMMY_LOGIT = -40.0                        # exp() underflows to 0 in f16

_compiled_cache = {}


def _build_graph(E_pad: int):
    import concourse.bacc as bacc
    import concourse.tile as tile
    from concourse import bass, mybir

    f32 = mybir.dt.float32
    f16 = mybir.dt.float16
    EPP = E_pad // 128                     # edges per partition
    NT = EPP // T                          # tiles
    CB = T // G                            # block cols per tile
    BCOL = EPP // G                        # block cols per partition
    BXW = BCOL + 2 * HB                    # halo'd b4 width

    nc = bacc.Bacc("TRN2", target_bir_lowering=False, debug=False,
                   num_devices=N_CORES)

    xs_d = nc.dram_tensor("xs", [E_pad, D], f16, kind="ExternalInput")
    w_d = nc.dram_tensor("wrep", [1, T * D], f16, kind="ExternalInput")
    b_d = nc.dram_tensor("bvec", [1, 1], f32, kind="ExternalInput")
    b02_d = nc.dram_tensor("b02", [1, 1], f32, kind="ExternalInput")
    v_d = nc.dram_tensor("vmem", [128, VW, BCOL], f16, kind="ExternalInput")
    out_d = nc.dram_tensor("out", [E_pad, 1], f32, kind="ExternalOutput")
    head_d = nc.dram_tensor("headb", [128, HB], f16)
    tail_d = nc.dram_tensor("tailb", [128, HB], f16)

    AP = bass.AP
    ALU = mybir.AluOpType
    ACT = mybir.ActivationFunctionType

    with tile.TileContext(nc) as tc:
        with (
            tc.tile_pool(name="consts", bufs=1) as consts,
            tc.tile_pool(name="xin", bufs=3) as xin,
            tc.tile_pool(name="prod", bufs=2) as prod,
            tc.tile_pool(name="small", bufs=4) as small,
            tc.tile_pool(name="keep", bufs=1) as keep,
            tc.tile_pool(name="bwork", bufs=2) as bwork,
        ):
            # --- constants ---
            wb = consts.tile([128, T, D], f16)
            nc.sync.dma_start(
                out=wb[:],
                in_=AP(tensor=w_d, offset=0, ap=[[0, 128], [1, T * D]]),
            )
            bb = consts.tile([128, 1], f32)
            nc.sync.dma_start(
                out=bb[:], in_=AP(tensor=b_d, offset=0, ap=[[0, 128], [1, 1]])
            )
            bb02 = consts.tile([128, 1], f32)
            nc.sync.dma_start(
                out=bb02[:], in_=AP(tensor=b02_d, offset=0, ap=[[0, 128], [1, 1]])
            )
            vmem = keep.tile([128, VW, BCOL], f16)
            nc.sync.dma_start(out=vmem[:], in_=v_d[:, :, :])

            e4 = keep.tile([128, EPP], f16)        # exp values, SBUF-resident
            b4x = keep.tile([128, BXW], f16)       # halo'd block sums

            # --- phase A: logits -> exp -> block sums ---
            with nc.allow_low_precision(
                reason="f16 accumulation: z err ~0.4%, budget 2e-2"
            ):
                for i in range(NT):
                    xt = xin.tile([128, T, D], f16)
                    nc.sync.dma_start(
                        out=xt[:],
                        in_=AP(tensor=xs_d, offset=i * T * D,
                               ap=[[EPP * D, 128], [D, T], [1, D]]),
                    )
                    pt = prod.tile([128, T, D], f16)
                    nc.vector.tensor_tensor(out=pt[:], in0=xt[:], in1=wb[:],
                                            op=ALU.mult)
                    zt = small.tile([128, T], f16, tag="zt")
                    nc.vector.tensor_reduce(out=zt[:], in_=pt[:],
                                            axis=mybir.AxisListType.X,
                                            op=ALU.add)
                    # leaky_relu(z+b) = 0.2*z + 0.2*b + 0.8*relu(z+b)
                    rt = small.tile([128, T], f16, tag="rt")
                    nc.scalar.activation(out=rt[:], in_=zt[:], func=ACT.Relu,
                                         bias=bb[:, 0:1], scale=1.0)
                    t1 = small.tile([128, T], f16, tag="t1")
                    nc.gpsimd.tensor_scalar(out=t1[:], in0=zt[:],
                                            scalar1=NEG_SLOPE,
                                            scalar2=bb02[:, 0:1],
                                            op0=ALU.mult, op1=ALU.add)
                    lt = small.tile([128, T], f16, tag="lt")
                    nc.gpsimd.tensor_scalar(out=lt[:], in0=rt[:],
                                            scalar1=1.0 - NEG_SLOPE,
                                            scalar2=None, op0=ALU.mult,
                                            accum_out=None)
                    lt2 = small.tile([128, T], f16, tag="lt2")
                    nc.gpsimd.tensor_tensor(out=lt2[:], in0=lt[:], in1=t1[:],
                                            op=ALU.add)
                    et = e4[:, i * T:(i + 1) * T]
                    nc.scalar.activation(out=et, in_=lt2[:], func=ACT.Exp)
                    nc.vector.tensor_reduce(
                        out=b4x[:, HB + i * CB:HB + (i + 1) * CB],
                        in_=et.rearrange("p (cb g) -> p cb g", g=G),
                        axis=mybir.AxisListType.X, op=ALU.add)

                # --- halo exchange via DRAM bounce ---
                # (engine ops can't start at partition >0: memset the full
                # halo regions, then DMA-overwrite the shifted partitions)
                nc.vector.memset(b4x[:, 0:HB], 0.0)
                nc.vector.memset(b4x[:, HB + BCOL:BXW], 0.0)
                nc.sync.dma_start(out=head_d[:, :], in_=b4x[:, HB:HB + HB])
                nc.sync.dma_start(out=tail_d[:, :],
                                  in_=b4x[:, BCOL:HB + BCOL])
                # left halo: partition p gets p-1's tail; partition 0 zeros
                nc.sync.dma_start(
                    out=b4x[1:128, 0:HB],
                    in_=AP(tensor=tail_d, offset=0, ap=[[HB, 127], [1, HB]]),
                )
                # right halo: partition p gets p+1's head; partition 127 zeros
                nc.sync.dma_start(
                    out=b4x[0:127, HB + BCOL:BXW],
                    in_=AP(tensor=head_d, offset=HB, ap=[[HB, 127], [1, HB]]),
                )

                # --- phase B: banded membership -> per-block denominators ---
                d4 = bwork.tile([128, BCOL], f16, tag="d4")
                nc.vector.tensor_tensor(out=d4[:], in0=b4x[:, 0:BCOL],
                                        in1=vmem[:, 0, :], op=ALU.mult)
                for v in range(1, VW):
                    tmp = bwork.tile([128, BCOL], f16, tag="btmp")
                    nc.vector.tensor_tensor(out=tmp[:], in0=b4x[:, v:v + BCOL],
                                            in1=vmem[:, v, :], op=ALU.mult)
                    nc.vector.tensor_tensor(out=d4[:], in0=d4[:], in1=tmp[:],
                                            op=ALU.add)

            d4f = bwork.tile([128, BCOL], f32, tag="d4f")
            nc.vector.tensor_scalar(out=d4f[:], in0=d4[:], scalar1=1e-6,
                                    scalar2=None, op0=ALU.add)
            r4 = bwork.tile([128, BCOL], f32, tag="r4")
            nc.vector.reciprocal(out=r4[:], in_=d4f[:])

            # --- phase C: out = e * R4[block] ---
            ot = bwork.tile([128, EPP], f32, tag="ot")
            r4_ap = r4[:]
            r4b = AP(tensor=r4_ap.tensor, offset=r4_ap.offset,
                     ap=[r4_ap.ap[0], r4_ap.ap[1], [0, G]])
            nc.vector.tensor_tensor(
                out=ot[:].rearrange("p (bc g) -> p bc g", g=G),
                in0=e4[:].rearrange("p (bc g) -> p bc g", g=G),
                in1=r4b, op=ALU.mult)
            nc.sync.dma_start(
                out=AP(tensor=out_d, offset=0, ap=[[EPP, 128], [1, EPP]]),
                in_=ot[:],
            )

    nc.compile()
    return nc


def _host_prep(x, W, b, index):
    """Sort/pad/shard on host; returns per-core in_maps plus reassembly info."""
    x = np.ascontiguousarray(np.asarray(x, dtype=np.float32))
    W = np.asarray(W, dtype=np.float32).reshape(D)
    b = np.asarray(b, dtype=np.float32).reshape(1)
    idx = np.asarray(index).astype(np.int64).ravel()
    E = idx.shape[0]

    order = np.argsort(idx, kind="stable")
    idx_s = idx[order]
    counts = np.bincount(idx_s, minlength=N_NODES).astype(np.int64)
    seg_starts = np.zeros(N_NODES + 1, dtype=np.int64)
    np.cumsum(counts, out=seg_starts[1:])
    plen = ((counts + G - 1) // G) * G                     # padded lengths
    assert plen.max() <= (HB + 1) * G, (
        f"segment of {plen.max()} padded edges exceeds band width {VW}"
    )

    core_e = seg_starts[np.arange(N_CORES + 1) * SEG_PER_CORE]
    pcum = np.zeros(N_NODES + 1, dtype=np.int64)
    np.cumsum(plen, out=pcum[1:])
    core_p = pcum[np.arange(N_CORES + 1) * SEG_PER_CORE]   # padded core bounds
    pcounts = np.diff(core_p)
    grain = 128 * T
    E_pad = int(np.ceil(max(pcounts.max(), 1) / grain) * grain)

    W16 = W.astype(np.float16)
    x_sorted16 = x[order].astype(np.float16)
    wrep = np.tile(W16, T).reshape(1, T * D)
    bvec = b.reshape(1, 1).astype(np.float32)
    b02 = (NEG_SLOPE * b).reshape(1, 1).astype(np.float32)
    wsq = float(W16.astype(np.float32) @ W16.astype(np.float32))
    dummy_row = ((DUMMY_LOGIT / max(wsq, 1e-30)) * W).astype(np.float16)

    NB = E_pad // G
    BCOL = NB // 128

    in_maps = []
    reasm = []
    for k in range(N_CORES):
        e0, e1 = int(core_e[k]), int(core_e[k + 1])
        cnt = e1 - e0
        s0 = k * SEG_PER_CORE
        scnt = counts[s0:s0 + SEG_PER_CORE]
        sstart = seg_starts[s0:s0 + SEG_PER_CORE] - e0     # compact local starts
        pstart = pcum[s0:s0 + SEG_PER_CORE] - int(core_p[k])  # padded local starts

        seg_local = (idx_s[e0:e1] - s0).astype(np.int64)
        pos_in_seg = np.arange(cnt, dtype=np.int64) - sstart[seg_local]
        ppos = pstart[seg_local] + pos_in_seg              # padded slot per edge

        xs = np.broadcast_to(dummy_row, (E_pad, D)).copy()
        xs[ppos] = x_sorted16[e0:e1]

        # block -> segment id (-1 for tail padding)
        nb = (plen[s0:s0 + SEG_PER_CORE] // G).astype(np.int64)
        bseg = np.full(NB, -1, dtype=np.int64)
        bseg[:int(nb.sum())] = np.repeat(np.arange(SEG_PER_CORE), nb)
        bpad = np.full(NB + 2 * HB, -2, dtype=np.int64)
        bpad[HB:HB + NB] = bseg
        V = np.empty((NB, VW), dtype=np.float16)
        for v in range(VW):
            V[:, v] = (bpad[v:v + NB] == bseg)
        vmem = np.ascontiguousarray(
            V.reshape(128, BCOL, VW).transpose(0, 2, 1))

        in_maps.append({
            "xs": xs, "wrep": wrep, "bvec": bvec, "b02": b02, "vmem": vmem,
        })
        reasm.append(ppos)

    return in_maps, reasm, order, core_e, E_pad, E


def _emulate_core(m, E_pad):
    """Numpy emulation of the device graph for one core (host-logic check)."""
    xs, wrep, bvec, b02 = m["xs"], m["wrep"], m["bvec"], m["b02"]
    vmem = m["vmem"]
    NB = E_pad // G
    z = (xs.astype(np.float32) @ wrep.ravel()[:D].astype(np.float32))
    r = np.maximum(z + bvec.ravel()[0], 0.0)
    l = NEG_SLOPE * z + b02.ravel()[0] + (1.0 - NEG_SLOPE) * r
    e = np.exp(l.astype(np.float16).astype(np.float32))
    e[l < -17.0] = 0.0                                     # f16 underflow
    e = e.astype(np.float32)
    # device edge order: partition p owns chunk [p*EPP, (p+1)*EPP) — same
    # linear order, so block ids are just position//G as before
    B4 = e.reshape(NB, G).sum(axis=1)
    B4p = np.concatenate([np.zeros(HB, np.float32), B4, np.zeros(HB, np.float32)])
    # vmem layout: [128, VW, BCOL], partition p owns blocks [p*BCOL, ...)
    BCOL = NB // 128
    V = vmem.transpose(0, 2, 1).reshape(NB, VW).astype(np.float32)
    D4 = np.zeros(NB, np.float32)
    for v in range(VW):
        D4 += B4p[v:v + NB] * V[:, v]
    R4 = 1.0 / (D4 + 1e-6)
    return (e * np.repeat(R4, G)).astype(np.float32)


LAST_RESULTS = None  # BassKernelResults from the most recent run


def kernel(x, W, b, index):
    global LAST_RESULTS
    in_maps, reasm, order, core_e, E_pad, E = _host_prep(x, W, b, index)

    if os.environ.get("KERNEL_EMULATE"):
        outs = [_emulate_core(m, E_pad) for m in in_maps]
    else:
        from concourse.bass_utils import run_bass_kernel_spmd

        if E_pad not in _compiled_cache:
            _compiled_cache[E_pad] = _build_graph(E_pad)
        nc = _compiled_cache[E_pad]
        trace = bool(os.environ.get("BASS_TRACE"))
        LAST_RESULTS = run_bass_kernel_spmd(
            nc, in_maps, list(range(N_CORES)), trace=trace,
        )
        outs = [r["out"] for r in LAST_RESULTS.results]

    out_sorted = np.empty(E, dtype=np.float32)
    for k in range(N_CORES):
        e0, e1 = int(core_e[k]), int(core_e[k + 1])
        out_sorted[e0:e1] = np.asarray(outs[k]).ravel()[reasm[k]]
    out = np.empty(E, dtype=np.float32)
    out[order] = out_sorted
    return out[:, None]


# revision 4
# speedup vs baseline: 2.5695x; 2.1030x over previous
"""Segment-softmax GNN attention kernel for 8 Trainium2 NeuronCores.

Math (reference): latent = leaky_relu(x @ W + b, 0.2)  -> [E, 1]
                  out = scatter_softmax(latent, index) -> [E, 1]

Strategy: host stable-sorts edges by destination segment, shards
segment-aligned across 8 cores (6250 segments each -> no cross-core
reduction), pads every segment to a multiple of G=4 edges (dummy rows
with logit -100 -> exp underflows to 0 in f16), converts x to float16
(halves DMA; ~0.1% error vs the 2e-2 budget), and stores it
TRANSPOSED [D, E_pad] so the feature dim lands on SBUF partitions with
perfectly contiguous DMA.

Device: the matvec runs entirely on the otherwise-idle TensorEngine.
Trick: the k-th matmul of a 128-matmul PSUM accumulation group uses a
sliding one-hot stationary (window k of a [128, 255] zero-padded W
band), so moving block k's dot products land in PSUM ROW k: each
accumulation group compacts 128 x n_g edges' logits into one [128,
n_g] PSUM tile with edges laid out partition-chunk-wise. DVE copies
PSUM->SBUF (f16), ScalarE applies exp(leaky) via the factorization
exp(0.2(z+b)) * exp(0.8 relu(z+b)), DVE forms per-4-edge block sums
directly in banded layout, then a halo exchange + banded 0/1
membership multiply (V, fp8) yields per-block softmax denominators.
out[e] = e[e] * R4[block(e)] in f32, one big store. Host drops padding
and inverse-permutes. Logits ~ N(0,1): no max-subtraction needed.
"""

import os
import sys

sys.path.insert(0, "/opt/trn_rl_repo")

import numpy as np

N_NODES = 50000
N_CORES = 8
SEG_PER_CORE = N_NODES // N_CORES          # 6250
D = 128
G = 4                                      # block granularity (segment pad)
NEG_SLOPE = 0.2
VW = 23                                    # membership band width (+-11 blocks)
HB = VW // 2
DUMMY_LOGIT = -100.0                       # exp(0.2*z) underflows to 0 in f16
NMAX = 432                                 # edges per PSUM row per group
KPT = 8                                    # matmuls per x DMA tile

_compiled_cache = {}


def _group_sizes(EPP: int) -> list:
    """Split the per-partition edge count into PSUM-group column counts:
    each <= NMAX, multiple of G."""
    ng = -(-EPP // NMAX)
    base = EPP // ng // G * G
    sizes = [base] * ng
    rem = EPP - base * ng
    assert rem % G == 0
    for i in range(rem // G):
        sizes[i % ng] += G
    assert sum(sizes) == EPP and all(s <= NMAX + G and s % G == 0 for s in sizes)
    return sizes


def _build_graph(E_pad: int):
    import concourse.bacc as bacc
    import concourse.tile as tile
    from concourse import bass, mybir

    f32 = mybir.dt.float32
    f16 = mybir.dt.float16
    fp8 = mybir.dt.float8e4
    EPP = E_pad // 128                     # edges per partition
    BCOL = EPP // G                        # block cols per partition
    BXW = BCOL + 2 * HB                    # halo'd b4 width
    nsz = _group_sizes(EPP)

    nc = bacc.Bacc("TRN2", target_bir_lowering=False, debug=False,
                   num_devices=N_CORES)

    xt_d = nc.dram_tensor("xst", [128, E_pad], f16, kind="ExternalInput")
    zp_d = nc.dram_tensor("zpad", [128, 255], f16, kind="ExternalInput")
    b02_d = nc.dram_tensor("b02", [1, 1], f32, kind="ExternalInput")
    b08_d = nc.dram_tensor("b08", [1, 1], f32, kind="ExternalInput")
    v_d = nc.dram_tensor("vmem", [128, VW, BCOL], fp8, kind="ExternalInput")
    out_d = nc.dram_tensor("out", [E_pad, 1], f32, kind="ExternalOutput")
    head_d = nc.dram_tensor("headb", [128, HB], f16)
    tail_d = nc.dram_tensor("tailb", [128, HB], f16)

    AP = bass.AP
    ALU = mybir.AluOpType
    ACT = mybir.ActivationFunctionType

    with tile.TileContext(nc) as tc:
        with (
            tc.tile_pool(name="consts", bufs=1) as consts,
            tc.tile_pool(name="xin", bufs=3) as xin,
            tc.tile_pool(name="small", bufs=3) as small,
            tc.tile_pool(name="keep", bufs=1) as keep,
            tc.tile_pool(name="bwork", bufs=2) as bwork,
            tc.tile_pool(name="psum", bufs=2,
                         space=bass.MemorySpace.PSUM) as psum,
        ):
            # --- constants ---
            zp = consts.tile([128, 255], f16)
            nc.sync.dma_start(out=zp[:], in_=zp_d[:, :])
            bb02 = consts.tile([128, 1], f32)
            nc.sync.dma_start(
                out=bb02[:], in_=AP(tensor=b02_d, offset=0, ap=[[0, 128], [1, 1]])
            )
            bb08 = consts.tile([128, 1], f32)
            nc.sync.dma_start(
                out=bb08[:], in_=AP(tensor=b08_d, offset=0, ap=[[0, 128], [1, 1]])
            )
            vmem = keep.tile([128, VW, BCOL], fp8)
            nc.sync.dma_start(out=vmem[:], in_=v_d[:, :, :])

            e4 = keep.tile([128, EPP], f16)        # exp values, SBUF-resident
            b4x = keep.tile([128, BXW], f16)       # halo'd block sums
            nc.vector.memset(b4x[:, 0:HB], 0.0)
            nc.vector.memset(b4x[:, HB + BCOL:BXW], 0.0)

            # --- phase A: PE matvec -> exp(leaky) -> block sums ---
            with nc.allow_low_precision(reason="f16 intermediates, 2e-2 budget"):
                roff = 0
                coff = 0
                for g, n in enumerate(nsz):
                    ps = psum.tile([128, NMAX + G], f32, tag="ps")
                    ntile = KPT * n
                    for t in range(128 // KPT):
                        xt = xin.tile([128, KPT * (NMAX + G)], f16, tag="xt")
                        nc.sync.dma_start(
                            out=xt[:, 0:ntile],
                            in_=AP(tensor=xt_d, offset=roff + t * ntile,
                                   ap=[[E_pad, 128], [1, ntile]]),
                        )
                        for j in range(KPT):
                            k = t * KPT + j
                            nc.tensor.matmul(
                                ps[:, 0:n],
                                zp[:, 127 - k:255 - k],
                                xt[:, j * n:(j + 1) * n],
                                start=(k == 0), stop=(k == 127),
                            )
                    roff += 128 * n

                    zs = small.tile([128, NMAX + G], f16, tag="zs")
                    nc.vector.tensor_copy(out=zs[:, 0:n], in_=ps[:, 0:n])
                    a1 = small.tile([128, NMAX + G], f16, tag="a1")
                    nc.scalar.activation(out=a1[:, 0:n], in_=zs[:, 0:n],
                                         func=ACT.Exp, bias=bb02[:, 0:1],
                                         scale=NEG_SLOPE)
                    rt = small.tile([128, NMAX + G], f16, tag="rt")
                    nc.scalar.activation(out=rt[:, 0:n], in_=zs[:, 0:n],
                                         func=ACT.Relu, bias=bb08[:, 0:1],
                                         scale=1.0 - NEG_SLOPE)
                    a2 = small.tile([128, NMAX + G], f16, tag="a2")
                    nc.scalar.activation(out=a2[:, 0:n], in_=rt[:, 0:n],
                                         func=ACT.Exp)
                    et = e4[:, coff:coff + n]
                    nc.vector.tensor_tensor(out=et, in0=a1[:, 0:n],
                                            in1=a2[:, 0:n], op=ALU.mult)
                    nc.vector.tensor_reduce(
                        out=b4x[:, HB + coff // G:HB + (coff + n) // G],
                        in_=et.rearrange("p (cb g) -> p cb g", g=G),
                        axis=mybir.AxisListType.X, op=ALU.add)
                    coff += n

                # --- halo exchange via DRAM bounce ---
                nc.sync.dma_start(out=head_d[:, :], in_=b4x[:, HB:HB + HB])
                nc.sync.dma_start(out=tail_d[:, :],
                                  in_=b4x[:, BCOL:HB + BCOL])
                # left halo: partition p gets p-1's tail; partition 0 zeros
                nc.sync.dma_start(
                    out=b4x[1:128, 0:HB],
                    in_=AP(tensor=tail_d, offset=0, ap=[[HB, 127], [1, HB]]),
                )
                # right halo: partition p gets p+1's head; partition 127 zeros
                nc.sync.dma_start(
                    out=b4x[0:127, HB + BCOL:BXW],
                    in_=AP(tensor=head_d, offset=HB, ap=[[HB, 127], [1, HB]]),
                )

                # --- phase B: banded membership -> per-block denominators ---
                d4 = bwork.tile([128, BCOL], f16, tag="d4")
                nc.vector.tensor_tensor(out=d4[:], in0=b4x[:, 0:BCOL],
                                        in1=vmem[:, 0, :], op=ALU.mult)
                for v in range(1, VW):
                    tmp = bwork.tile([128, BCOL], f16, tag="btmp")
                    nc.vector.tensor_tensor(out=tmp[:], in0=b4x[:, v:v + BCOL],
                                            in1=vmem[:, v, :], op=ALU.mult)
                    nc.vector.tensor_tensor(out=d4[:], in0=d4[:], in1=tmp[:],
                                            op=ALU.add)

            d4f = bwork.tile([128, BCOL], f32, tag="d4f")
            nc.vector.tensor_scalar(out=d4f[:], in0=d4[:], scalar1=1e-6,
                                    scalar2=None, op0=ALU.add)
            r4 = bwork.tile([128, BCOL], f32, tag="r4")
            nc.vector.reciprocal(out=r4[:], in_=d4f[:])

            # --- phase C: out = e * R4[block] ---
            ot = bwork.tile([128, EPP], f32, tag="ot")
            r4_ap = r4[:]
            r4b = AP(tensor=r4_ap.tensor, offset=r4_ap.offset,
                     ap=[r4_ap.ap[0], r4_ap.ap[1], [0, G]])
            nc.vector.tensor_tensor(
                out=ot[:].rearrange("p (bc g) -> p bc g", g=G),
                in0=e4[:].rearrange("p (bc g) -> p bc g", g=G),
                in1=r4b, op=ALU.mult)
            nc.sync.dma_start(
                out=AP(tensor=out_d, offset=0, ap=[[EPP, 128], [1, EPP]]),
                in_=ot[:],
            )

    nc.compile()
    return nc


def _dram_col_of_q(E_pad: int):
    """Map linear padded-edge position q -> column r in the transposed
    DRAM tensor, such that PE group/row streaming lands edge q at SBUF
    (partition q//EPP, column q%EPP)."""
    EPP = E_pad // 128
    nsz = np.array(_group_sizes(EPP), dtype=np.int64)
    coff = np.zeros(len(nsz) + 1, dtype=np.int64)
    np.cumsum(nsz, out=coff[1:])
    q = np.arange(E_pad, dtype=np.int64)
    p = q // EPP
    c = q % EPP
    g = np.searchsorted(coff, c, side="right") - 1
    return 128 * coff[g] + p * nsz[g] + (c - coff[g])


def _host_prep(x, W, b, index):
    """Sort/pad/shard on host; returns per-core in_maps plus reassembly info."""
    import ml_dtypes

    x = np.ascontiguousarray(np.asarray(x, dtype=np.float32))
    W = np.asarray(W, dtype=np.float32).reshape(D)
    b = np.asarray(b, dtype=np.float32).reshape(1)
    idx = np.asarray(index).astype(np.int64).ravel()
    E = idx.shape[0]

    order = np.argsort(idx, kind="stable")
    idx_s = idx[order]
    counts = np.bincount(idx_s, minlength=N_NODES).astype(np.int64)
    seg_starts = np.zeros(N_NODES + 1, dtype=np.int64)
    np.cumsum(counts, out=seg_starts[1:])
    plen = ((counts + G - 1) // G) * G                     # padded lengths
    assert plen.max() <= (HB + 1) * G, (
        f"segment of {plen.max()} padded edges exceeds band width {VW}"
    )

    core_e = seg_starts[np.arange(N_CORES + 1) * SEG_PER_CORE]
    pcum = np.zeros(N_NODES + 1, dtype=np.int64)
    np.cumsum(plen, out=pcum[1:])
    core_p = pcum[np.arange(N_CORES + 1) * SEG_PER_CORE]   # padded core bounds
    pcounts = np.diff(core_p)
    E_pad = int(np.ceil(max(pcounts.max(), 1) / 512) * 512)

    W16 = W.astype(np.float16)
    x_sorted16 = x[order].astype(np.float16)
    zpad = np.zeros((128, 255), dtype=np.float16)
    zpad[:, 127] = W16
    b02 = (NEG_SLOPE * b).reshape(1, 1).astype(np.float32)
    b08 = ((1.0 - NEG_SLOPE) * b).reshape(1, 1).astype(np.float32)
    wsq = float(W16.astype(np.float32) @ W16.astype(np.float32))
    dummy_row = ((DUMMY_LOGIT / max(wsq, 1e-30)) * W).astype(np.float16)

    NB = E_pad // G
    BCOL = NB // 128
    r_of_q = _dram_col_of_q(E_pad)

    in_maps = []
    reasm = []
    for k in range(N_CORES):
        e0, e1 = int(core_e[k]), int(core_e[k + 1])
        cnt = e1 - e0
        s0 = k * SEG_PER_CORE
        sstart = seg_starts[s0:s0 + SEG_PER_CORE] - e0     # compact local starts
        pstart = pcum[s0:s0 + SEG_PER_CORE] - int(core_p[k])  # padded local starts

        seg_local = (idx_s[e0:e1] - s0).astype(np.int64)
        pos_in_seg = np.arange(cnt, dtype=np.int64) - sstart[seg_local]
        ppos = pstart[seg_local] + pos_in_seg              # padded slot per edge

        xst = np.tile(dummy_row[:, None], (1, E_pad))      # [128, E_pad] f16
        xst[:, r_of_q[ppos]] = x_sorted16[e0:e1].T

        # block -> segment id (-1 for tail padding)
        nb = (plen[s0:s0 + SEG_PER_CORE] // G).astype(np.int64)
        bseg = np.full(NB, -1, dtype=np.int64)
        bseg[:int(nb.sum())] = np.repeat(np.arange(SEG_PER_CORE), nb)
        bpad = np.full(NB + 2 * HB, -2, dtype=np.int64)
        bpad[HB:HB + NB] = bseg
        V = np.empty((NB, VW), dtype=ml_dtypes.float8_e4m3fn)
        for v in range(VW):
            V[:, v] = (bpad[v:v + NB] == bseg).astype(np.float32)
        vmem = np.ascontiguousarray(
            V.reshape(128, BCOL, VW).transpose(0, 2, 1))

        in_maps.append({
            "xst": xst, "zpad": zpad, "b02": b02, "b08": b08, "vmem": vmem,
        })
        reasm.append(ppos)

    return in_maps, reasm, order, core_e, E_pad, E


def _emulate_core(m, E_pad):
    """Numpy emulation of the device graph for one core (host-logic check)."""
    xst, zpad, b02, b08 = m["xst"], m["zpad"], m["b02"], m["b08"]
    vmem = m["vmem"]
    NB = E_pad // G
    r_of_q = _dram_col_of_q(E_pad)
    xp = xst[:, r_of_q].T.astype(np.float32)               # [E_pad, 128]
    W = zpad[:, 127].astype(np.float32)
    z = (xp @ W).astype(np.float16).astype(np.float32)
    a1 = np.exp(NEG_SLOPE * z + b02.ravel()[0])
    a1[NEG_SLOPE * z + b02.ravel()[0] < -17.0] = 0.0       # f16 underflow
    a2 = np.exp(np.maximum((1 - NEG_SLOPE) * z + b08.ravel()[0], 0.0))
    e = (a1 * a2).astype(np.float32)
    B4 = e.reshape(NB, G).sum(axis=1)
    B4p = np.concatenate([np.zeros(HB, np.float32), B4, np.zeros(HB, np.float32)])
    BCOL = NB // 128
    V = vmem.transpose(0, 2, 1).reshape(NB, VW).astype(np.float32)
    D4 = np.zeros(NB, np.float32)
    for v in range(VW):
        D4 += B4p[v:v + NB] * V[:, v]
    R4 = 1.0 / (D4 + 1e-6)
    return (e * np.repeat(R4, G)).astype(np.float32)


LAST_RESULTS = None  # BassKernelResults from the most recent run


def kernel(x, W, b, index):
    global LAST_RESULTS
    in_maps, reasm, order, core_e, E_pad, E = _host_prep(x, W, b, index)

    if os.environ.get("KERNEL_EMULATE"):
        outs = [_emulate_core(m, E_pad) for m in in_maps]
    else:
        from concourse.bass_utils import run_bass_kernel_spmd

        if E_pad not in _compiled_cache:
            _compiled_cache[E_pad] = _build_graph(E_pad)
        nc = _compiled_cache[E_pad]
        trace = bool(os.environ.get("BASS_TRACE"))
        LAST_RESULTS = run_bass_kernel_spmd(
            nc, in_maps, list(range(N_CORES)), trace=trace,
        )
        outs = [r["out"] for r in LAST_RESULTS.results]

    out_sorted = np.empty(E, dtype=np.float32)
    for k in range(N_CORES):
        e0, e1 = int(core_e[k]), int(core_e[k + 1])
        out_sorted[e0:e1] = np.asarray(outs[k]).ravel()[reasm[k]]
    out = np.empty(E, dtype=np.float32)
    out[order] = out_sorted
    return out[:, None]


# revision 11
# speedup vs baseline: 2.7590x; 1.0738x over previous
"""Segment-softmax GNN attention kernel for 8 Trainium2 NeuronCores.

Math (reference): latent = leaky_relu(x @ W + b, 0.2)  -> [E, 1]
                  out = scatter_softmax(latent, index) -> [E, 1]

Strategy: host stable-sorts edges by destination segment, shards
segment-aligned across 8 cores (6250 segments each -> no cross-core
reduction), pads every segment to a multiple of G=4 edges (dummy rows
with logit -100 -> exp underflows to 0 in f16), converts x to float16
(halves DMA; ~0.1% error vs the 2e-2 budget), and stores it
TRANSPOSED [D, E_pad] so the feature dim lands on SBUF partitions with
perfectly contiguous DMA.

Device: the matvec runs entirely on the otherwise-idle TensorEngine.
Trick: the k-th matmul of a 128-matmul PSUM accumulation group uses a
sliding one-hot stationary (window k of a zero-padded W band), so
moving block k's dot products land in PSUM ROW k: each group compacts
128 x n_g edge logits into one [128, n_g] PSUM bank with edges in
partition-chunk layout. DVE copies PSUM->SBUF (f16), ScalarE applies
exp(leaky) via exp(0.2(z+b)) * exp(0.8 relu(z+b)), DVE forms per-4-edge
block sums directly in banded layout. Groups stream LAST-first so both
partition-halo exchanges complete early; the banded 0/1-membership
denominator pass (V, fp8), reciprocal, e*R multiply and output store
then run INCREMENTALLY per settled block-column range, overlapped with
the x stream — only the final group's slice remains in the tail.
Host drops padding and inverse-permutes. Logits ~ N(0,1): no
max-subtraction needed.
"""

import os
import sys

sys.path.insert(0, "/opt/trn_rl_repo")

import numpy as np

N_NODES = 50000
N_CORES = 8
SEG_PER_CORE = N_NODES // N_CORES          # 6250
D = 128
G = 4                                      # block granularity (segment pad)
NEG_SLOPE = 0.2
VW = 23                                    # membership band width (+-11 blocks)
HB = VW // 2
DUMMY_LOGIT = -100.0                       # exp(0.2*z) underflows to 0 in f16
NMAX = 216                                 # max edges per PSUM row per group
KPT = 8                                    # matmuls per x DMA tile

_compiled_cache = {}


def _group_sizes(EPP: int) -> list:
    """Split the per-partition edge count into PSUM-group column counts:
    each <= NMAX+G, multiple of G."""
    ng = -(-EPP // NMAX)
    base = EPP // ng // G * G
    sizes = [base] * ng
    rem = EPP - base * ng
    assert rem % G == 0
    for i in range(rem // G):
        sizes[i % ng] += G
    assert sum(sizes) == EPP and all(s % G == 0 for s in sizes)
    return sizes


def _stream_order(ng: int) -> list:
    """Process the last group first so the left partition halo (previous
    partition's tail block-sums) is available early."""
    return [ng - 1] + list(range(ng - 1))


def _build_graph(E_pad: int):
    import concourse.bacc as bacc
    import concourse.tile as tile
    from concourse import bass, mybir

    f32 = mybir.dt.float32
    f16 = mybir.dt.float16
    fp8 = mybir.dt.float8e4
    EPP = E_pad // 128                     # edges per partition
    BCOL = EPP // G                        # block cols per partition
    BXW = BCOL + 2 * HB                    # halo'd b4 width
    nsz = _group_sizes(EPP)
    NG = len(nsz)
    coff = [0]
    for n in nsz:
        coff.append(coff[-1] + n)

    nc = bacc.Bacc("TRN2", target_bir_lowering=False, debug=False,
                   num_devices=N_CORES)

    xt_d = nc.dram_tensor("xst", [128, E_pad], f16, kind="ExternalInput")
    zp_d = nc.dram_tensor("zpad", [128, 255], f16, kind="ExternalInput")
    b02_d = nc.dram_tensor("b02", [1, 1], f32, kind="ExternalInput")
    b08_d = nc.dram_tensor("b08", [1, 1], f32, kind="ExternalInput")
    v_d = nc.dram_tensor("vmem", [128, VW, BCOL], fp8, kind="ExternalInput")
    out_d = nc.dram_tensor("out", [E_pad, 1], f16, kind="ExternalOutput")
    head_d = nc.dram_tensor("headb", [128, HB], f16)
    tail_d = nc.dram_tensor("tailb", [128, HB], f16)

    AP = bass.AP
    ALU = mybir.AluOpType
    ACT = mybir.ActivationFunctionType

    with tile.TileContext(nc) as tc:
        with (
            tc.tile_pool(name="consts", bufs=1) as consts,
            tc.tile_pool(name="xin", bufs=6) as xin,
            tc.tile_pool(name="small", bufs=3) as small,
            tc.tile_pool(name="keep", bufs=1) as keep,
            tc.tile_pool(name="bwork", bufs=2) as bwork,
            tc.tile_pool(name="psum", bufs=2,
                         space=bass.MemorySpace.PSUM) as psum,
        ):
            # --- constants (scalar queue, so x streaming starts first) ---
            zp = consts.tile([128, 255], f16)
            nc.scalar.dma_start(out=zp[:], in_=zp_d[:, :])
            bb02 = consts.tile([128, 1], f32)
            nc.scalar.dma_start(
                out=bb02[:], in_=AP(tensor=b02_d, offset=0, ap=[[0, 128], [1, 1]])
            )
            bb08 = consts.tile([128, 1], f32)
            nc.scalar.dma_start(
                out=bb08[:], in_=AP(tensor=b08_d, offset=0, ap=[[0, 128], [1, 1]])
            )
            vmem = keep.tile([128, VW, BCOL], fp8)
            nc.scalar.dma_start(out=vmem[:], in_=v_d[:, :, :])

            e4 = keep.tile([128, EPP], f16)        # exp values, SBUF-resident
            b4x = keep.tile([128, BXW], f16)       # halo'd block sums
            d4 = keep.tile([128, BCOL], f16)       # denominators per block
            r4 = keep.tile([128, BCOL], f32)       # reciprocals
            nc.vector.memset(b4x[:, 0:HB], 0.0)
            nc.vector.memset(b4x[:, HB + BCOL:BXW], 0.0)

            # coverage of halo'd block-col coords [0, BXW) for readiness
            covered = np.zeros(BXW, dtype=bool)
            emitted = np.zeros(BCOL, dtype=bool)

            def emit_ready_chunks(final: bool):
                ready = np.ones(BCOL, dtype=bool)
                for c in range(BCOL):
                    ready[c] = covered[c:c + VW].all()
                ready &= ~emitted
                if final:
                    assert ready.any() or emitted.all()
                idx = np.flatnonzero(ready)
                if idx.size == 0:
                    return
                splits = np.flatnonzero(np.diff(idx) > 1)
                runs = np.split(idx, splits + 1)
                for run in runs:
                    c0, c1 = int(run[0]), int(run[-1]) + 1
                    emitted[c0:c1] = True
                    _banded_chunk(c0, c1)

            def _banded_chunk(c0, c1):
                w = c1 - c0
                # denominators: band over halo'd block sums
                nc.vector.tensor_tensor(out=d4[:, c0:c1],
                                        in0=b4x[:, c0:c1],
                                        in1=vmem[:, 0, c0:c1], op=ALU.mult)
                for v in range(1, VW):
                    tmp = bwork.tile([128, BCOL], f16, tag="btmp")
                    nc.vector.tensor_tensor(out=tmp[:, 0:w],
                                            in0=b4x[:, c0 + v:c1 + v],
                                            in1=vmem[:, v, c0:c1], op=ALU.mult)
                    nc.vector.tensor_tensor(out=d4[:, c0:c1],
                                            in0=d4[:, c0:c1],
                                            in1=tmp[:, 0:w], op=ALU.add)
                d4f = bwork.tile([128, BCOL], f32, tag="d4f")
                nc.vector.tensor_scalar(out=d4f[:, 0:w], in0=d4[:, c0:c1],
                                        scalar1=1e-6, scalar2=None,
                                        op0=ALU.add)
                nc.vector.reciprocal(out=r4[:, c0:c1], in_=d4f[:, 0:w])
                # out = e * R4[block], f16, store this column range
                ot = bwork.tile([128, 512], f16, tag="ot")
                rc = r4[:, c0:c1]
                r4b = AP(tensor=rc.tensor, offset=rc.offset,
                         ap=[rc.ap[0], rc.ap[1], [0, G]])
                nc.vector.tensor_tensor(
                    out=ot[:, 0:w * G].rearrange("p (bc g) -> p bc g", g=G),
                    in0=e4[:, c0 * G:c1 * G].rearrange("p (bc g) -> p bc g",
                                                       g=G),
                    in1=r4b, op=ALU.mult)
                nc.sync.dma_start(
                    out=AP(tensor=out_d, offset=c0 * G,
                           ap=[[EPP, 128], [1, w * G]]),
                    in_=ot[:, 0:w * G],
                )

            # --- phase A: PE matvec -> exp(leaky) -> block sums,
            #     with incremental banded softmax chunks ---
            with nc.allow_low_precision(reason="f16 intermediates, 2e-2 budget"):
                roff = 0
                for g in _stream_order(NG):
                    n = nsz[g]
                    c_lo, c_hi = coff[g], coff[g + 1]
                    ps = psum.tile([128, 512], f32, tag="ps")  # one full bank
                    ntile = KPT * n
                    for t in range(128 // KPT):
                        xt = xin.tile([128, KPT * (NMAX + G)], f16, tag="xt")
                        nc.sync.dma_start(
                            out=xt[:, 0:ntile],
                            in_=AP(tensor=xt_d, offset=roff + t * ntile,
                                   ap=[[E_pad, 128], [1, ntile]]),
                        )
                        for j in range(KPT):
                            k = t * KPT + j
                            s = k % 128
                            nc.tensor.matmul(
                                ps[:, 0:n],
                                zp[:, 127 - s:255 - s],
                                xt[:, j * n:(j + 1) * n],
                                start=(k == 0), stop=(k == 127),
                            )
                    roff += 128 * n

                    zs = small.tile([128, NMAX + G], f16, tag="zs")
                    nc.vector.tensor_copy(out=zs[:, 0:n], in_=ps[:, 0:n])
                    a1 = small.tile([128, NMAX + G], f16, tag="a1")
                    nc.scalar.activation(out=a1[:, 0:n], in_=zs[:, 0:n],
                                         func=ACT.Exp, bias=bb02[:, 0:1],
                                         scale=NEG_SLOPE)
                    rt = small.tile([128, NMAX + G], f16, tag="rt")
                    nc.scalar.activation(out=rt[:, 0:n], in_=zs[:, 0:n],
                                         func=ACT.Relu, bias=bb08[:, 0:1],
                                         scale=1.0 - NEG_SLOPE)
                    a2 = small.tile([128, NMAX + G], f16, tag="a2")
                    nc.scalar.activation(out=a2[:, 0:n], in_=rt[:, 0:n],
                                         func=ACT.Exp)
                    et = e4[:, c_lo:c_hi]
                    nc.vector.tensor_tensor(out=et, in0=a1[:, 0:n],
                                            in1=a2[:, 0:n], op=ALU.mult)
                    nc.vector.tensor_reduce(
                        out=b4x[:, HB + c_lo // G:HB + c_hi // G],
                        in_=et.rearrange("p (cb g) -> p cb g", g=G),
                        axis=mybir.AxisListType.X, op=ALU.add)
                    covered[HB + c_lo // G:HB + c_hi // G] = True

                    if g == NG - 1:
                        # streamed first: tail block-sums ready -> left halo
                        nc.sync.dma_start(out=tail_d[:, :],
                                          in_=b4x[:, BCOL:HB + BCOL])
                        nc.sync.dma_start(
                            out=b4x[1:128, 0:HB],
                            in_=AP(tensor=tail_d, offset=0,
                                   ap=[[HB, 127], [1, HB]]),
                        )
                        covered[0:HB] = True
                    if g == 0:
                        # head block-sums ready -> right halo
                        nc.sync.dma_start(out=head_d[:, :],
                                          in_=b4x[:, HB:HB + HB])
                        nc.sync.dma_start(
                            out=b4x[0:127, HB + BCOL:BXW],
                            in_=AP(tensor=head_d, offset=HB,
                                   ap=[[HB, 127], [1, HB]]),
                        )
                        covered[HB + BCOL:BXW] = True

                    emit_ready_chunks(final=False)
                emit_ready_chunks(final=True)

    nc.compile()
    return nc


def _dram_col_of_q(E_pad: int):
    """Map linear padded-edge position q -> column r in the transposed
    DRAM tensor, such that PE group/row streaming lands edge q at SBUF
    (partition q//EPP, column q%EPP)."""
    EPP = E_pad // 128
    nsz = np.array(_group_sizes(EPP), dtype=np.int64)
    coff = np.zeros(len(nsz) + 1, dtype=np.int64)
    np.cumsum(nsz, out=coff[1:])
    roff = np.zeros(len(nsz), dtype=np.int64)
    acc = 0
    for g in _stream_order(len(nsz)):
        roff[g] = acc
        acc += 128 * int(nsz[g])
    q = np.arange(E_pad, dtype=np.int64)
    p = q // EPP
    c = q % EPP
    g = np.searchsorted(coff, c, side="right") - 1
    return roff[g] + p * nsz[g] + (c - coff[g])


def _host_prep(x, W, b, index):
    """Sort/pad/shard on host; returns per-core in_maps plus reassembly info."""
    import ml_dtypes

    x = np.ascontiguousarray(np.asarray(x, dtype=np.float32))
    W = np.asarray(W, dtype=np.float32).reshape(D)
    b = np.asarray(b, dtype=np.float32).reshape(1)
    idx = np.asarray(index).astype(np.int64).ravel()
    E = idx.shape[0]

    order = np.argsort(idx, kind="stable")
    idx_s = idx[order]
    counts = np.bincount(idx_s, minlength=N_NODES).astype(np.int64)
    seg_starts = np.zeros(N_NODES + 1, dtype=np.int64)
    np.cumsum(counts, out=seg_starts[1:])
    plen = ((counts + G - 1) // G) * G                     # padded lengths
    assert plen.max() <= (HB + 1) * G, (
        f"segment of {plen.max()} padded edges exceeds band width {VW}"
    )

    core_e = seg_starts[np.arange(N_CORES + 1) * SEG_PER_CORE]
    pcum = np.zeros(N_NODES + 1, dtype=np.int64)
    np.cumsum(plen, out=pcum[1:])
    core_p = pcum[np.arange(N_CORES + 1) * SEG_PER_CORE]   # padded core bounds
    pcounts = np.diff(core_p)
    E_pad = int(np.ceil(max(pcounts.max(), 1) / 512) * 512)

    W16 = W.astype(np.float16)
    zpad = np.zeros((128, 255), dtype=np.float16)
    zpad[:, 127] = W16
    x_sorted16 = x[order].astype(np.float16)
    b02 = (NEG_SLOPE * b).reshape(1, 1).astype(np.float32)
    b08 = ((1.0 - NEG_SLOPE) * b).reshape(1, 1).astype(np.float32)
    wsq = float(W16.astype(np.float32) @ W16.astype(np.float32))
    dummy_row = ((DUMMY_LOGIT / max(wsq, 1e-30)) * W).astype(np.float16)

    NB = E_pad // G
    BCOL = NB // 128
    r_of_q = _dram_col_of_q(E_pad)

    in_maps = []
    reasm = []
    for k in range(N_CORES):
        e0, e1 = int(core_e[k]), int(core_e[k + 1])
        cnt = e1 - e0
        s0 = k * SEG_PER_CORE
        sstart = seg_starts[s0:s0 + SEG_PER_CORE] - e0     # compact local starts
        pstart = pcum[s0:s0 + SEG_PER_CORE] - int(core_p[k])  # padded local starts

        seg_local = (idx_s[e0:e1] - s0).astype(np.int64)
        pos_in_seg = np.arange(cnt, dtype=np.int64) - sstart[seg_local]
        ppos = pstart[seg_local] + pos_in_seg              # padded slot per edge

        xst = np.tile(dummy_row[:, None], (1, E_pad))      # [128, E_pad] f16
        xst[:, r_of_q[ppos]] = x_sorted16[e0:e1].T

        # block -> segment id (-1 for tail padding)
        nb = (plen[s0:s0 + SEG_PER_CORE] // G).astype(np.int64)
        bseg = np.full(NB, -1, dtype=np.int64)
        bseg[:int(nb.sum())] = np.repeat(np.arange(SEG_PER_CORE), nb)
        bpad = np.full(NB + 2 * HB, -2, dtype=np.int64)
        bpad[HB:HB + NB] = bseg
        V = np.empty((NB, VW), dtype=ml_dtypes.float8_e4m3fn)
        for v in range(VW):
            V[:, v] = (bpad[v:v + NB] == bseg).astype(np.float32)
        vmem = np.ascontiguousarray(
            V.reshape(128, BCOL, VW).transpose(0, 2, 1))

        in_maps.append({
            "xst": xst, "zpad": zpad, "b02": b02, "b08": b08, "vmem": vmem,
        })
        reasm.append(ppos)

    return in_maps, reasm, order, core_e, E_pad, E


def _emulate_core(m, E_pad):
    """Numpy emulation of the device graph for one core (host-logic check)."""
    xst, zpad, b02, b08 = m["xst"], m["zpad"], m["b02"], m["b08"]
    vmem = m["vmem"]
    NB = E_pad // G
    r_of_q = _dram_col_of_q(E_pad)
    xp = xst[:, r_of_q].T.astype(np.float32)               # [E_pad, 128]
    W = zpad[:, 127].astype(np.float32)
    z = (xp @ W).astype(np.float16).astype(np.float32)
    a1 = np.exp(NEG_SLOPE * z + b02.ravel()[0])
    a1[NEG_SLOPE * z + b02.ravel()[0] < -17.0] = 0.0       # f16 underflow
    a2 = np.exp(np.maximum((1 - NEG_SLOPE) * z + b08.ravel()[0], 0.0))
    e = (a1 * a2).astype(np.float32)
    B4 = e.reshape(NB, G).sum(axis=1)
    B4p = np.concatenate([np.zeros(HB, np.float32), B4, np.zeros(HB, np.float32)])
    BCOL = NB // 128
    V = vmem.transpose(0, 2, 1).reshape(NB, VW).astype(np.float32)
    D4 = np.zeros(NB, np.float32)
    for v in range(VW):
        D4 += B4p[v:v + NB] * V[:, v]
    R4 = 1.0 / (D4 + 1e-6)
    return (e * np.repeat(R4, G)).astype(np.float16).astype(np.float32)


LAST_RESULTS = None  # BassKernelResults from the most recent run


def kernel(x, W, b, index):
    global LAST_RESULTS
    in_maps, reasm, order, core_e, E_pad, E = _host_prep(x, W, b, index)

    if os.environ.get("KERNEL_EMULATE"):
        outs = [_emulate_core(m, E_pad) for m in in_maps]
    else:
        from concourse.bass_utils import run_bass_kernel_spmd

        if E_pad not in _compiled_cache:
            _compiled_cache[E_pad] = _build_graph(E_pad)
        nc = _compiled_cache[E_pad]
        trace = bool(os.environ.get("BASS_TRACE"))
        LAST_RESULTS = run_bass_kernel_spmd(
            nc, in_maps, list(range(N_CORES)), trace=trace,
        )
        outs = [r["out"] for r in LAST_RESULTS.results]

    out_sorted = np.empty(E, dtype=np.float32)
    for k in range(N_CORES):
        e0, e1 = int(core_e[k]), int(core_e[k + 1])
        out_sorted[e0:e1] = np.asarray(outs[k]).astype(np.float32).ravel()[reasm[k]]
    out = np.empty(E, dtype=np.float32)
    out[order] = out_sorted
    return out[:, None]


# revision 12
# speedup vs baseline: 2.8214x; 1.0226x over previous
"""Segment-softmax GNN attention kernel for 8 Trainium2 NeuronCores.

Math (reference): latent = leaky_relu(x @ W + b, 0.2)  -> [E, 1]
                  out = scatter_softmax(latent, index) -> [E, 1]

Strategy: host stable-sorts edges by destination segment, shards
segment-aligned across 8 cores (6250 segments each -> no cross-core
reduction), pads every segment to a multiple of G=4 edges (dummy rows
with logit -100 -> exp underflows to 0 in f16), converts x to float16
(halves DMA; ~0.1% error vs the 2e-2 budget), and stores it
TRANSPOSED [D, E_pad] so the feature dim lands on SBUF partitions with
perfectly contiguous DMA.

Device: the matvec runs entirely on the otherwise-idle TensorEngine.
Trick: the k-th matmul of a 128-matmul PSUM accumulation group uses a
sliding one-hot stationary (window k of a zero-padded W band), so
moving block k's dot products land in PSUM ROW k: each group compacts
128 x n_g edge logits into one [128, n_g] PSUM bank with edges in
partition-chunk layout. DVE copies PSUM->SBUF (f16), ScalarE applies
exp(leaky) via exp(0.2(z+b)) * exp(0.8 relu(z+b)), DVE forms per-4-edge
block sums directly in banded layout. Groups stream LAST-first so both
partition-halo exchanges complete early; the banded 0/1-membership
denominator pass (V, fp8), reciprocal, e*R multiply and output store
then run INCREMENTALLY per settled block-column range, overlapped with
the x stream — only the final group's slice remains in the tail.
Host drops padding and inverse-permutes. Logits ~ N(0,1): no
max-subtraction needed.
"""

import os
import sys

sys.path.insert(0, "/opt/trn_rl_repo")

import numpy as np

N_NODES = 50000
N_CORES = 8
SEG_PER_CORE = N_NODES // N_CORES          # 6250
D = 128
G = 4                                      # block granularity (segment pad)
NEG_SLOPE = 0.2
VW = 23                                    # membership band width (+-11 blocks)
HB = VW // 2
DUMMY_LOGIT = -100.0                       # exp(0.2*z) underflows to 0 in f16
NMAX = 512                                 # max edges per PSUM row per group
KPT = 8                                    # matmuls per x DMA tile

_compiled_cache = {}


def _group_sizes(EPP: int) -> list:
    """Split the per-partition edge count into PSUM-group column counts
    (each <= 512, multiple of G). A 512-wide group keeps LDWEIGHTS hidden
    under matmuls; a small group at index NG-2 is streamed LAST so the
    final banded-softmax chunk is narrow."""
    assert EPP % G == 0
    if EPP <= 512:
        return [EPP]
    rem = EPP - 512
    if rem <= 128:
        return [512, rem]
    small = 116
    rest = rem - small
    sizes = [512, small]
    while rest > 512:
        sizes.append(512)
        rest -= 512
    sizes.append(rest)
    assert sum(sizes) == EPP and all(4 <= s <= 512 and s % G == 0
                                     for s in sizes)
    return sizes


def _stream_order(ng: int) -> list:
    """Process the last group first so the left partition halo (previous
    partition's tail block-sums) is available early; group NG-2 (the small
    one) lands last."""
    return [ng - 1] + list(range(ng - 1))


def _build_graph(E_pad: int):
    import concourse.bacc as bacc
    import concourse.tile as tile
    from concourse import bass, mybir

    f32 = mybir.dt.float32
    f16 = mybir.dt.float16
    fp8 = mybir.dt.float8e4
    EPP = E_pad // 128                     # edges per partition
    BCOL = EPP // G                        # block cols per partition
    BXW = BCOL + 2 * HB                    # halo'd b4 width
    nsz = _group_sizes(EPP)
    NG = len(nsz)
    coff = [0]
    for n in nsz:
        coff.append(coff[-1] + n)

    nc = bacc.Bacc("TRN2", target_bir_lowering=False, debug=False,
                   num_devices=N_CORES)

    xt_d = nc.dram_tensor("xst", [128, E_pad], f16, kind="ExternalInput")
    zp_d = nc.dram_tensor("zpad", [128, 255], f16, kind="ExternalInput")
    b02_d = nc.dram_tensor("b02", [1, 1], f32, kind="ExternalInput")
    b08_d = nc.dram_tensor("b08", [1, 1], f32, kind="ExternalInput")
    v_d = nc.dram_tensor("vmem", [128, VW, BCOL], fp8, kind="ExternalInput")
    out_d = nc.dram_tensor("out", [E_pad, 1], f16, kind="ExternalOutput")
    head_d = nc.dram_tensor("headb", [128, HB], f16)
    tail_d = nc.dram_tensor("tailb", [128, HB], f16)

    AP = bass.AP
    ALU = mybir.AluOpType
    ACT = mybir.ActivationFunctionType

    with tile.TileContext(nc) as tc:
        with (
            tc.tile_pool(name="consts", bufs=1) as consts,
            tc.tile_pool(name="xin", bufs=6) as xin,
            tc.tile_pool(name="small", bufs=3) as small,
            tc.tile_pool(name="keep", bufs=1) as keep,
            tc.tile_pool(name="bwork", bufs=2) as bwork,
            tc.tile_pool(name="psum", bufs=2,
                         space=bass.MemorySpace.PSUM) as psum,
        ):
            # --- constants (scalar queue, so x streaming starts first) ---
            zp = consts.tile([128, 255], f16)
            nc.scalar.dma_start(out=zp[:], in_=zp_d[:, :])
            bb02 = consts.tile([128, 1], f32)
            nc.scalar.dma_start(
                out=bb02[:], in_=AP(tensor=b02_d, offset=0, ap=[[0, 128], [1, 1]])
            )
            bb08 = consts.tile([128, 1], f32)
            nc.scalar.dma_start(
                out=bb08[:], in_=AP(tensor=b08_d, offset=0, ap=[[0, 128], [1, 1]])
            )
            vmem = keep.tile([128, VW, BCOL], fp8)
            nc.scalar.dma_start(out=vmem[:], in_=v_d[:, :, :])

            e4 = keep.tile([128, EPP], f16)        # exp values, SBUF-resident
            b4x = keep.tile([128, BXW], f16)       # halo'd block sums
            d4 = keep.tile([128, BCOL], f16)       # denominators per block
            r4 = keep.tile([128, BCOL], f32)       # reciprocals
            nc.vector.memset(b4x[:, 0:HB], 0.0)
            nc.vector.memset(b4x[:, HB + BCOL:BXW], 0.0)

            # coverage of halo'd block-col coords [0, BXW) for readiness
            covered = np.zeros(BXW, dtype=bool)
            emitted = np.zeros(BCOL, dtype=bool)

            def emit_ready_chunks(final: bool):
                ready = np.ones(BCOL, dtype=bool)
                for c in range(BCOL):
                    ready[c] = covered[c:c + VW].all()
                ready &= ~emitted
                if final:
                    assert ready.any() or emitted.all()
                idx = np.flatnonzero(ready)
                if idx.size == 0:
                    return
                splits = np.flatnonzero(np.diff(idx) > 1)
                runs = np.split(idx, splits + 1)
                for run in runs:
                    c0, c1 = int(run[0]), int(run[-1]) + 1
                    emitted[c0:c1] = True
                    _banded_chunk(c0, c1)

            def _banded_chunk(c0, c1):
                w = c1 - c0
                # denominators: band over halo'd block sums
                nc.vector.tensor_tensor(out=d4[:, c0:c1],
                                        in0=b4x[:, c0:c1],
                                        in1=vmem[:, 0, c0:c1], op=ALU.mult)
                for v in range(1, VW):
                    tmp = bwork.tile([128, BCOL], f16, tag="btmp")
                    nc.vector.tensor_tensor(out=tmp[:, 0:w],
                                            in0=b4x[:, c0 + v:c1 + v],
                                            in1=vmem[:, v, c0:c1], op=ALU.mult)
                    nc.vector.tensor_tensor(out=d4[:, c0:c1],
                                            in0=d4[:, c0:c1],
                                            in1=tmp[:, 0:w], op=ALU.add)
                d4f = bwork.tile([128, BCOL], f32, tag="d4f")
                nc.vector.tensor_scalar(out=d4f[:, 0:w], in0=d4[:, c0:c1],
                                        scalar1=1e-6, scalar2=None,
                                        op0=ALU.add)
                nc.vector.reciprocal(out=r4[:, c0:c1], in_=d4f[:, 0:w])
                # out = e * R4[block], f16, store this column range
                ot = bwork.tile([128, 512], f16, tag="ot")
                rc = r4[:, c0:c1]
                r4b = AP(tensor=rc.tensor, offset=rc.offset,
                         ap=[rc.ap[0], rc.ap[1], [0, G]])
                nc.vector.tensor_tensor(
                    out=ot[:, 0:w * G].rearrange("p (bc g) -> p bc g", g=G),
                    in0=e4[:, c0 * G:c1 * G].rearrange("p (bc g) -> p bc g",
                                                       g=G),
                    in1=r4b, op=ALU.mult)
                nc.sync.dma_start(
                    out=AP(tensor=out_d, offset=c0 * G,
                           ap=[[EPP, 128], [1, w * G]]),
                    in_=ot[:, 0:w * G],
                )

            # --- phase A: PE matvec -> exp(leaky) -> block sums,
            #     with incremental banded softmax chunks ---
            with nc.allow_low_precision(reason="f16 intermediates, 2e-2 budget"):
                roff = 0
                for g in _stream_order(NG):
                    n = nsz[g]
                    c_lo, c_hi = coff[g], coff[g + 1]
                    ps = psum.tile([128, 512], f32, tag="ps")  # one full bank
                    ntile = KPT * n
                    for t in range(128 // KPT):
                        xt = xin.tile([128, KPT * (NMAX + G)], f16, tag="xt")
                        nc.sync.dma_start(
                            out=xt[:, 0:ntile],
                            in_=AP(tensor=xt_d, offset=roff + t * ntile,
                                   ap=[[E_pad, 128], [1, ntile]]),
                        )
                        for j in range(KPT):
                            k = t * KPT + j
                            s = k % 128
                            nc.tensor.matmul(
                                ps[:, 0:n],
                                zp[:, 127 - s:255 - s],
                                xt[:, j * n:(j + 1) * n],
                                start=(k == 0), stop=(k == 127),
                            )
                    roff += 128 * n

                    zs = small.tile([128, NMAX + G], f16, tag="zs")
                    nc.vector.tensor_copy(out=zs[:, 0:n], in_=ps[:, 0:n])
                    a1 = small.tile([128, NMAX + G], f16, tag="a1")
                    nc.scalar.activation(out=a1[:, 0:n], in_=zs[:, 0:n],
                                         func=ACT.Exp, bias=bb02[:, 0:1],
                                         scale=NEG_SLOPE)
                    rt = small.tile([128, NMAX + G], f16, tag="rt")
                    nc.scalar.activation(out=rt[:, 0:n], in_=zs[:, 0:n],
                                         func=ACT.Relu, bias=bb08[:, 0:1],
                                         scale=1.0 - NEG_SLOPE)
                    a2 = small.tile([128, NMAX + G], f16, tag="a2")
                    nc.scalar.activation(out=a2[:, 0:n], in_=rt[:, 0:n],
                                         func=ACT.Exp)
                    et = e4[:, c_lo:c_hi]
                    nc.vector.tensor_tensor(out=et, in0=a1[:, 0:n],
                                            in1=a2[:, 0:n], op=ALU.mult)
                    nc.vector.tensor_reduce(
                        out=b4x[:, HB + c_lo // G:HB + c_hi // G],
                        in_=et.rearrange("p (cb g) -> p cb g", g=G),
                        axis=mybir.AxisListType.X, op=ALU.add)
                    covered[HB + c_lo // G:HB + c_hi // G] = True

                    if g == NG - 1:
                        # streamed first: tail block-sums ready -> left halo
                        nc.sync.dma_start(out=tail_d[:, :],
                                          in_=b4x[:, BCOL:HB + BCOL])
                        nc.sync.dma_start(
                            out=b4x[1:128, 0:HB],
                            in_=AP(tensor=tail_d, offset=0,
                                   ap=[[HB, 127], [1, HB]]),
                        )
                        covered[0:HB] = True
                    if g == 0:
                        # head block-sums ready -> right halo
                        nc.sync.dma_start(out=head_d[:, :],
                                          in_=b4x[:, HB:HB + HB])
                        nc.sync.dma_start(
                            out=b4x[0:127, HB + BCOL:BXW],
                            in_=AP(tensor=head_d, offset=HB,
                                   ap=[[HB, 127], [1, HB]]),
                        )
                        covered[HB + BCOL:BXW] = True

                    emit_ready_chunks(final=False)
                emit_ready_chunks(final=True)

    nc.compile()
    return nc


def _dram_col_of_q(E_pad: int):
    """Map linear padded-edge position q -> column r in the transposed
    DRAM tensor, such that PE group/row streaming lands edge q at SBUF
    (partition q//EPP, column q%EPP)."""
    EPP = E_pad // 128
    nsz = np.array(_group_sizes(EPP), dtype=np.int64)
    coff = np.zeros(len(nsz) + 1, dtype=np.int64)
    np.cumsum(nsz, out=coff[1:])
    roff = np.zeros(len(nsz), dtype=np.int64)
    acc = 0
    for g in _stream_order(len(nsz)):
        roff[g] = acc
        acc += 128 * int(nsz[g])
    q = np.arange(E_pad, dtype=np.int64)
    p = q // EPP
    c = q % EPP
    g = np.searchsorted(coff, c, side="right") - 1
    return roff[g] + p * nsz[g] + (c - coff[g])


def _host_prep(x, W, b, index):
    """Sort/pad/shard on host; returns per-core in_maps plus reassembly info."""
    import ml_dtypes

    x = np.ascontiguousarray(np.asarray(x, dtype=np.float32))
    W = np.asarray(W, dtype=np.float32).reshape(D)
    b = np.asarray(b, dtype=np.float32).reshape(1)
    idx = np.asarray(index).astype(np.int64).ravel()
    E = idx.shape[0]

    order = np.argsort(idx, kind="stable")
    idx_s = idx[order]
    counts = np.bincount(idx_s, minlength=N_NODES).astype(np.int64)
    seg_starts = np.zeros(N_NODES + 1, dtype=np.int64)
    np.cumsum(counts, out=seg_starts[1:])
    plen = ((counts + G - 1) // G) * G                     # padded lengths
    assert plen.max() <= (HB + 1) * G, (
        f"segment of {plen.max()} padded edges exceeds band width {VW}"
    )

    core_e = seg_starts[np.arange(N_CORES + 1) * SEG_PER_CORE]
    pcum = np.zeros(N_NODES + 1, dtype=np.int64)
    np.cumsum(plen, out=pcum[1:])
    core_p = pcum[np.arange(N_CORES + 1) * SEG_PER_CORE]   # padded core bounds
    pcounts = np.diff(core_p)
    E_pad = int(np.ceil(max(pcounts.max(), 1) / 512) * 512)

    W16 = W.astype(np.float16)
    zpad = np.zeros((128, 255), dtype=np.float16)
    zpad[:, 127] = W16
    x_sorted16 = x[order].astype(np.float16)
    b02 = (NEG_SLOPE * b).reshape(1, 1).astype(np.float32)
    b08 = ((1.0 - NEG_SLOPE) * b).reshape(1, 1).astype(np.float32)
    wsq = float(W16.astype(np.float32) @ W16.astype(np.float32))
    dummy_row = ((DUMMY_LOGIT / max(wsq, 1e-30)) * W).astype(np.float16)

    NB = E_pad // G
    BCOL = NB // 128
    r_of_q = _dram_col_of_q(E_pad)

    in_maps = []
    reasm = []
    for k in range(N_CORES):
        e0, e1 = int(core_e[k]), int(core_e[k + 1])
        cnt = e1 - e0
        s0 = k * SEG_PER_CORE
        sstart = seg_starts[s0:s0 + SEG_PER_CORE] - e0     # compact local starts
        pstart = pcum[s0:s0 + SEG_PER_CORE] - int(core_p[k])  # padded local starts

        seg_local = (idx_s[e0:e1] - s0).astype(np.int64)
        pos_in_seg = np.arange(cnt, dtype=np.int64) - sstart[seg_local]
        ppos = pstart[seg_local] + pos_in_seg              # padded slot per edge

        xst = np.tile(dummy_row[:, None], (1, E_pad))      # [128, E_pad] f16
        xst[:, r_of_q[ppos]] = x_sorted16[e0:e1].T

        # block -> segment id (-1 for tail padding)
        nb = (plen[s0:s0 + SEG_PER_CORE] // G).astype(np.int64)
        bseg = np.full(NB, -1, dtype=np.int64)
        bseg[:int(nb.sum())] = np.repeat(np.arange(SEG_PER_CORE), nb)
        bpad = np.full(NB + 2 * HB, -2, dtype=np.int64)
        bpad[HB:HB + NB] = bseg
        V = np.empty((NB, VW), dtype=ml_dtypes.float8_e4m3fn)
        for v in range(VW):
            V[:, v] = (bpad[v:v + NB] == bseg).astype(np.float32)
        vmem = np.ascontiguousarray(
            V.reshape(128, BCOL, VW).transpose(0, 2, 1))

        in_maps.append({
            "xst": xst, "zpad": zpad, "b02": b02, "b08": b08, "vmem": vmem,
        })
        reasm.append(ppos)

    return in_maps, reasm, order, core_e, E_pad, E


def _emulate_core(m, E_pad):
    """Numpy emulation of the device graph for one core (host-logic check)."""
    xst, zpad, b02, b08 = m["xst"], m["zpad"], m["b02"], m["b08"]
    vmem = m["vmem"]
    NB = E_pad // G
    r_of_q = _dram_col_of_q(E_pad)
    xp = xst[:, r_of_q].T.astype(np.float32)               # [E_pad, 128]
    W = zpad[:, 127].astype(np.float32)
    z = (xp @ W).astype(np.float16).astype(np.float32)
    a1 = np.exp(NEG_SLOPE * z + b02.ravel()[0])
    a1[NEG_SLOPE * z + b02.ravel()[0] < -17.0] = 0.0       # f16 underflow
    a2 = np.exp(np.maximum((1 - NEG_SLOPE) * z + b08.ravel()[0], 0.0))
    e = (a1 * a2).astype(np.float32)
    B4 = e.reshape(NB, G).sum(axis=1)
    B4p = np.concatenate([np.zeros(HB, np.float32), B4, np.zeros(HB, np.float32)])
    BCOL = NB // 128
    V = vmem.transpose(0, 2, 1).reshape(NB, VW).astype(np.float32)
    D4 = np.zeros(NB, np.float32)
    for v in range(VW):
        D4 += B4p[v:v + NB] * V[:, v]
    R4 = 1.0 / (D4 + 1e-6)
    return (e * np.repeat(R4, G)).astype(np.float16).astype(np.float32)


LAST_RESULTS = None  # BassKernelResults from the most recent run


def kernel(x, W, b, index):
    global LAST_RESULTS
    in_maps, reasm, order, core_e, E_pad, E = _host_prep(x, W, b, index)

    if os.environ.get("KERNEL_EMULATE"):
        outs = [_emulate_core(m, E_pad) for m in in_maps]
    else:
        from concourse.bass_utils import run_bass_kernel_spmd

        if E_pad not in _compiled_cache:
            _compiled_cache[E_pad] = _build_graph(E_pad)
        nc = _compiled_cache[E_pad]
        trace = bool(os.environ.get("BASS_TRACE"))
        LAST_RESULTS = run_bass_kernel_spmd(
            nc, in_maps, list(range(N_CORES)), trace=trace,
        )
        outs = [r["out"] for r in LAST_RESULTS.results]

    out_sorted = np.empty(E, dtype=np.float32)
    for k in range(N_CORES):
        e0, e1 = int(core_e[k]), int(core_e[k + 1])
        out_sorted[e0:e1] = np.asarray(outs[k]).astype(np.float32).ravel()[reasm[k]]
    out = np.empty(E, dtype=np.float32)
    out[order] = out_sorted
    return out[:, None]


# revision 13
# speedup vs baseline: 2.9208x; 1.0352x over previous
"""Segment-softmax GNN attention kernel for 8 Trainium2 NeuronCores.

Math (reference): latent = leaky_relu(x @ W + b, 0.2)  -> [E, 1]
                  out = scatter_softmax(latent, index) -> [E, 1]

Strategy: host stable-sorts edges by destination segment, shards
segment-aligned across 8 cores (6250 segments each -> no cross-core
reduction), pads every segment to a multiple of G=4 edges (dummy rows
with logit -100 -> exp underflows to 0 in f16), converts x to float16
(halves DMA; ~0.1% error vs the 2e-2 budget), and stores it
TRANSPOSED [D, E_pad] so the feature dim lands on SBUF partitions with
perfectly contiguous DMA.

Device: the matvec runs entirely on the otherwise-idle TensorEngine.
Trick: the k-th matmul of a 128-matmul PSUM accumulation group uses a
sliding one-hot stationary (window k of a zero-padded W band), so
moving block k's dot products land in PSUM ROW k: each group compacts
128 x n_g edge logits into one [128, n_g] PSUM bank with edges in
partition-chunk layout. DVE copies PSUM->SBUF (f16), ScalarE applies
exp(leaky) via exp(0.2(z+b)) * exp(0.8 relu(z+b)), DVE forms per-4-edge
block sums directly in banded layout. Groups stream LAST-first so both
partition-halo exchanges complete early; the banded 0/1-membership
denominator pass (V, fp8), reciprocal, e*R multiply and output store
then run INCREMENTALLY per settled block-column range, overlapped with
the x stream — only the final group's slice remains in the tail.
Host drops padding and inverse-permutes. Logits ~ N(0,1): no
max-subtraction needed.
"""

import os
import sys

sys.path.insert(0, "/opt/trn_rl_repo")

import numpy as np

N_NODES = 50000
N_CORES = 8
SEG_PER_CORE = N_NODES // N_CORES          # 6250
D = 128
G = 4                                      # block granularity (segment pad)
NEG_SLOPE = 0.2
VW = 23                                    # membership band width (+-11 blocks)
HB = VW // 2
DUMMY_LOGIT = -100.0                       # exp(0.2*z) underflows to 0 in f16
NMAX = 512                                 # max edges per PSUM row per group
KPT = 8                                    # matmuls per x DMA tile

_compiled_cache = {}


def _group_sizes(EPP: int) -> list:
    """Split the per-partition edge count into PSUM-group column counts
    (each <= 512, multiple of G). A 512-wide group keeps LDWEIGHTS hidden
    under matmuls; a small group at index NG-2 is streamed LAST so the
    final banded-softmax chunk is narrow."""
    assert EPP % G == 0
    if EPP <= 512:
        return [EPP]
    rem = EPP - 512
    if rem <= 128:
        return [512, rem]
    small = 116
    rest = rem - small
    sizes = [512, small]
    while rest > 512:
        sizes.append(512)
        rest -= 512
    sizes.append(rest)
    assert sum(sizes) == EPP and all(4 <= s <= 512 and s % G == 0
                                     for s in sizes)
    return sizes


def _stream_order(ng: int) -> list:
    """Process the last group first so the left partition halo (previous
    partition's tail block-sums) is available early; group NG-2 (the small
    one) lands last."""
    return [ng - 1] + list(range(ng - 1))


def _build_graph(E_pad: int):
    import concourse.bacc as bacc
    import concourse.tile as tile
    from concourse import bass, mybir

    f32 = mybir.dt.float32
    f16 = mybir.dt.float16
    fp8 = mybir.dt.float8e4
    EPP = E_pad // 128                     # edges per partition
    BCOL = EPP // G                        # block cols per partition
    BXW = BCOL + 2 * HB                    # halo'd b4 width
    nsz = _group_sizes(EPP)
    NG = len(nsz)
    coff = [0]
    for n in nsz:
        coff.append(coff[-1] + n)

    nc = bacc.Bacc("TRN2", target_bir_lowering=False, debug=False,
                   num_devices=N_CORES)

    xt_d = nc.dram_tensor("xst", [128, E_pad], f16, kind="ExternalInput")
    zp_d = nc.dram_tensor("zpad", [128, 255], f16, kind="ExternalInput")
    b02_d = nc.dram_tensor("b02", [1, 1], f32, kind="ExternalInput")
    b08_d = nc.dram_tensor("b08", [1, 1], f32, kind="ExternalInput")
    v_d = nc.dram_tensor("vmem", [128, VW, BCOL], fp8, kind="ExternalInput")
    out_d = nc.dram_tensor("out", [E_pad, 1], f16, kind="ExternalOutput")
    head_d = nc.dram_tensor("headb", [128, HB], f16)
    tail_d = nc.dram_tensor("tailb", [128, HB], f16)

    AP = bass.AP
    ALU = mybir.AluOpType
    ACT = mybir.ActivationFunctionType

    with tile.TileContext(nc) as tc:
        with (
            tc.tile_pool(name="consts", bufs=1) as consts,
            tc.tile_pool(name="xin", bufs=6) as xin,
            tc.tile_pool(name="small", bufs=3) as small,
            tc.tile_pool(name="keep", bufs=1) as keep,
            tc.tile_pool(name="bwork", bufs=2) as bwork,
            tc.tile_pool(name="psum", bufs=2,
                         space=bass.MemorySpace.PSUM) as psum,
        ):
            # --- constants (scalar queue, so x streaming starts first) ---
            zp = consts.tile([128, 255], f16)
            nc.scalar.dma_start(out=zp[:], in_=zp_d[:, :])
            bb02 = consts.tile([128, 1], f32)
            nc.scalar.dma_start(
                out=bb02[:], in_=AP(tensor=b02_d, offset=0, ap=[[0, 128], [1, 1]])
            )
            bb08 = consts.tile([128, 1], f32)
            nc.scalar.dma_start(
                out=bb08[:], in_=AP(tensor=b08_d, offset=0, ap=[[0, 128], [1, 1]])
            )
            vmem = keep.tile([128, VW, BCOL], fp8)
            nc.scalar.dma_start(out=vmem[:], in_=v_d[:, :, :])

            e4 = keep.tile([128, EPP], f16)        # exp values, SBUF-resident
            b4x = keep.tile([128, BXW], f16)       # halo'd block sums
            d4 = keep.tile([128, BCOL], f16)       # denominators per block
            r4 = keep.tile([128, BCOL], f32)       # reciprocals
            nc.vector.memset(b4x[:, 0:HB], 0.0)
            nc.vector.memset(b4x[:, HB + BCOL:BXW], 0.0)

            # coverage of halo'd block-col coords [0, BXW) for readiness
            covered = np.zeros(BXW, dtype=bool)
            emitted = np.zeros(BCOL, dtype=bool)

            def emit_ready_chunks(final: bool):
                ready = np.ones(BCOL, dtype=bool)
                for c in range(BCOL):
                    ready[c] = covered[c:c + VW].all()
                ready &= ~emitted
                if final:
                    assert ready.any() or emitted.all()
                idx = np.flatnonzero(ready)
                if idx.size == 0:
                    return
                splits = np.flatnonzero(np.diff(idx) > 1)
                runs = np.split(idx, splits + 1)
                for run in runs:
                    c0, c1 = int(run[0]), int(run[-1]) + 1
                    emitted[c0:c1] = True
                    _banded_chunk(c0, c1)

            def _banded_chunk(c0, c1):
                w = c1 - c0
                # denominators: band over halo'd block sums
                nc.vector.tensor_tensor(out=d4[:, c0:c1],
                                        in0=b4x[:, c0:c1],
                                        in1=vmem[:, 0, c0:c1], op=ALU.mult)
                for v in range(1, VW):
                    tmp = bwork.tile([128, BCOL], f16, tag="btmp")
                    nc.vector.tensor_tensor(out=tmp[:, 0:w],
                                            in0=b4x[:, c0 + v:c1 + v],
                                            in1=vmem[:, v, c0:c1], op=ALU.mult)
                    nc.vector.tensor_tensor(out=d4[:, c0:c1],
                                            in0=d4[:, c0:c1],
                                            in1=tmp[:, 0:w], op=ALU.add)
                d4f = bwork.tile([128, BCOL], f32, tag="d4f")
                nc.vector.tensor_scalar(out=d4f[:, 0:w], in0=d4[:, c0:c1],
                                        scalar1=1e-6, scalar2=None,
                                        op0=ALU.add)
                nc.vector.reciprocal(out=r4[:, c0:c1], in_=d4f[:, 0:w])
                # out = e * R4[block], f16, store this column range
                ot = bwork.tile([128, 512], f16, tag="ot")
                rc = r4[:, c0:c1]
                r4b = AP(tensor=rc.tensor, offset=rc.offset,
                         ap=[rc.ap[0], rc.ap[1], [0, G]])
                nc.vector.tensor_tensor(
                    out=ot[:, 0:w * G].rearrange("p (bc g) -> p bc g", g=G),
                    in0=e4[:, c0 * G:c1 * G].rearrange("p (bc g) -> p bc g",
                                                       g=G),
                    in1=r4b, op=ALU.mult)
                nc.scalar.dma_start(
                    out=AP(tensor=out_d, offset=c0 * G,
                           ap=[[EPP, 128], [1, w * G]]),
                    in_=ot[:, 0:w * G],
                )

            # --- phase A: PE matvec -> exp(leaky) -> block sums,
            #     with incremental banded softmax chunks ---
            with nc.allow_low_precision(reason="f16 intermediates, 2e-2 budget"):
                roff = 0
                for g in _stream_order(NG):
                    n = nsz[g]
                    c_lo, c_hi = coff[g], coff[g + 1]
                    ps = psum.tile([128, 512], f32, tag="ps")  # one full bank
                    ntile = KPT * n
                    for t in range(128 // KPT):
                        xt = xin.tile([128, KPT * (NMAX + G)], f16, tag="xt")
                        nc.sync.dma_start(
                            out=xt[:, 0:ntile],
                            in_=AP(tensor=xt_d, offset=roff + t * ntile,
                                   ap=[[E_pad, 128], [1, ntile]]),
                        )
                        for j in range(KPT):
                            k = t * KPT + j
                            s = k % 128
                            nc.tensor.matmul(
                                ps[:, 0:n],
                                zp[:, 127 - s:255 - s],
                                xt[:, j * n:(j + 1) * n],
                                start=(k == 0), stop=(k == 127),
                            )
                    roff += 128 * n

                    zs = small.tile([128, NMAX + G], f16, tag="zs")
                    nc.vector.tensor_copy(out=zs[:, 0:n], in_=ps[:, 0:n])
                    a1 = small.tile([128, NMAX + G], f16, tag="a1")
                    nc.scalar.activation(out=a1[:, 0:n], in_=zs[:, 0:n],
                                         func=ACT.Exp, bias=bb02[:, 0:1],
                                         scale=NEG_SLOPE)
                    rt = small.tile([128, NMAX + G], f16, tag="rt")
                    nc.scalar.activation(out=rt[:, 0:n], in_=zs[:, 0:n],
                                         func=ACT.Relu, bias=bb08[:, 0:1],
                                         scale=1.0 - NEG_SLOPE)
                    a2 = small.tile([128, NMAX + G], f16, tag="a2")
                    nc.scalar.activation(out=a2[:, 0:n], in_=rt[:, 0:n],
                                         func=ACT.Exp)
                    et = e4[:, c_lo:c_hi]
                    nc.vector.tensor_tensor(out=et, in0=a1[:, 0:n],
                                            in1=a2[:, 0:n], op=ALU.mult)
                    nc.vector.tensor_reduce(
                        out=b4x[:, HB + c_lo // G:HB + c_hi // G],
                        in_=et.rearrange("p (cb g) -> p cb g", g=G),
                        axis=mybir.AxisListType.X, op=ALU.add)
                    covered[HB + c_lo // G:HB + c_hi // G] = True

                    if g == NG - 1:
                        # streamed first: tail block-sums ready -> left halo
                        nc.scalar.dma_start(out=tail_d[:, :],
                                            in_=b4x[:, BCOL:HB + BCOL])
                        nc.scalar.dma_start(
                            out=b4x[1:128, 0:HB],
                            in_=AP(tensor=tail_d, offset=0,
                                   ap=[[HB, 127], [1, HB]]),
                        )
                        covered[0:HB] = True
                    if g == 0:
                        # head block-sums ready -> right halo
                        nc.scalar.dma_start(out=head_d[:, :],
                                            in_=b4x[:, HB:HB + HB])
                        nc.scalar.dma_start(
                            out=b4x[0:127, HB + BCOL:BXW],
                            in_=AP(tensor=head_d, offset=HB,
                                   ap=[[HB, 127], [1, HB]]),
                        )
                        covered[HB + BCOL:BXW] = True

                    emit_ready_chunks(final=False)
                emit_ready_chunks(final=True)

    nc.compile()
    return nc


def _dram_col_of_q(E_pad: int):
    """Map linear padded-edge position q -> column r in the transposed
    DRAM tensor, such that PE group/row streaming lands edge q at SBUF
    (partition q//EPP, column q%EPP)."""
    EPP = E_pad // 128
    nsz = np.array(_group_sizes(EPP), dtype=np.int64)
    coff = np.zeros(len(nsz) + 1, dtype=np.int64)
    np.cumsum(nsz, out=coff[1:])
    roff = np.zeros(len(nsz), dtype=np.int64)
    acc = 0
    for g in _stream_order(len(nsz)):
        roff[g] = acc
        acc += 128 * int(nsz[g])
    q = np.arange(E_pad, dtype=np.int64)
    p = q // EPP
    c = q % EPP
    g = np.searchsorted(coff, c, side="right") - 1
    return roff[g] + p * nsz[g] + (c - coff[g])


def _host_prep(x, W, b, index):
    """Sort/pad/shard on host; returns per-core in_maps plus reassembly info."""
    import ml_dtypes

    x = np.ascontiguousarray(np.asarray(x, dtype=np.float32))
    W = np.asarray(W, dtype=np.float32).reshape(D)
    b = np.asarray(b, dtype=np.float32).reshape(1)
    idx = np.asarray(index).astype(np.int64).ravel()
    E = idx.shape[0]

    order = np.argsort(idx, kind="stable")
    idx_s = idx[order]
    counts = np.bincount(idx_s, minlength=N_NODES).astype(np.int64)
    seg_starts = np.zeros(N_NODES + 1, dtype=np.int64)
    np.cumsum(counts, out=seg_starts[1:])
    plen = ((counts + G - 1) // G) * G                     # padded lengths
    assert plen.max() <= (HB + 1) * G, (
        f"segment of {plen.max()} padded edges exceeds band width {VW}"
    )

    core_e = seg_starts[np.arange(N_CORES + 1) * SEG_PER_CORE]
    pcum = np.zeros(N_NODES + 1, dtype=np.int64)
    np.cumsum(plen, out=pcum[1:])
    core_p = pcum[np.arange(N_CORES + 1) * SEG_PER_CORE]   # padded core bounds
    pcounts = np.diff(core_p)
    E_pad = int(np.ceil(max(pcounts.max(), 1) / 512) * 512)

    W16 = W.astype(np.float16)
    zpad = np.zeros((128, 255), dtype=np.float16)
    zpad[:, 127] = W16
    x_sorted16 = x[order].astype(np.float16)
    b02 = (NEG_SLOPE * b).reshape(1, 1).astype(np.float32)
    b08 = ((1.0 - NEG_SLOPE) * b).reshape(1, 1).astype(np.float32)
    wsq = float(W16.astype(np.float32) @ W16.astype(np.float32))
    dummy_row = ((DUMMY_LOGIT / max(wsq, 1e-30)) * W).astype(np.float16)

    NB = E_pad // G
    BCOL = NB // 128
    r_of_q = _dram_col_of_q(E_pad)

    in_maps = []
    reasm = []
    for k in range(N_CORES):
        e0, e1 = int(core_e[k]), int(core_e[k + 1])
        cnt = e1 - e0
        s0 = k * SEG_PER_CORE
        sstart = seg_starts[s0:s0 + SEG_PER_CORE] - e0     # compact local starts
        pstart = pcum[s0:s0 + SEG_PER_CORE] - int(core_p[k])  # padded local starts

        seg_local = (idx_s[e0:e1] - s0).astype(np.int64)
        pos_in_seg = np.arange(cnt, dtype=np.int64) - sstart[seg_local]
        ppos = pstart[seg_local] + pos_in_seg              # padded slot per edge

        xst = np.tile(dummy_row[:, None], (1, E_pad))      # [128, E_pad] f16
        xst[:, r_of_q[ppos]] = x_sorted16[e0:e1].T

        # block -> segment id (-1 for tail padding)
        nb = (plen[s0:s0 + SEG_PER_CORE] // G).astype(np.int64)
        bseg = np.full(NB, -1, dtype=np.int64)
        bseg[:int(nb.sum())] = np.repeat(np.arange(SEG_PER_CORE), nb)
        bpad = np.full(NB + 2 * HB, -2, dtype=np.int64)
        bpad[HB:HB + NB] = bseg
        V = np.empty((NB, VW), dtype=ml_dtypes.float8_e4m3fn)
        for v in range(VW):
            V[:, v] = (bpad[v:v + NB] == bseg).astype(np.float32)
        vmem = np.ascontiguousarray(
            V.reshape(128, BCOL, VW).transpose(0, 2, 1))

        in_maps.append({
            "xst": xst, "zpad": zpad, "b02": b02, "b08": b08, "vmem": vmem,
        })
        reasm.append(ppos)

    return in_maps, reasm, order, core_e, E_pad, E


def _emulate_core(m, E_pad):
    """Numpy emulation of the device graph for one core (host-logic check)."""
    xst, zpad, b02, b08 = m["xst"], m["zpad"], m["b02"], m["b08"]
    vmem = m["vmem"]
    NB = E_pad // G
    r_of_q = _dram_col_of_q(E_pad)
    xp = xst[:, r_of_q].T.astype(np.float32)               # [E_pad, 128]
    W = zpad[:, 127].astype(np.float32)
    z = (xp @ W).astype(np.float16).astype(np.float32)
    a1 = np.exp(NEG_SLOPE * z + b02.ravel()[0])
    a1[NEG_SLOPE * z + b02.ravel()[0] < -17.0] = 0.0       # f16 underflow
    a2 = np.exp(np.maximum((1 - NEG_SLOPE) * z + b08.ravel()[0], 0.0))
    e = (a1 * a2).astype(np.float32)
    B4 = e.reshape(NB, G).sum(axis=1)
    B4p = np.concatenate([np.zeros(HB, np.float32), B4, np.zeros(HB, np.float32)])
    BCOL = NB // 128
    V = vmem.transpose(0, 2, 1).reshape(NB, VW).astype(np.float32)
    D4 = np.zeros(NB, np.float32)
    for v in range(VW):
        D4 += B4p[v:v + NB] * V[:, v]
    R4 = 1.0 / (D4 + 1e-6)
    return (e * np.repeat(R4, G)).astype(np.float16).astype(np.float32)


LAST_RESULTS = None  # BassKernelResults from the most recent run


def kernel(x, W, b, index):
    global LAST_RESULTS
    in_maps, reasm, order, core_e, E_pad, E = _host_prep(x, W, b, index)

    if os.environ.get("KERNEL_EMULATE"):
        outs = [_emulate_core(m, E_pad) for m in in_maps]
    else:
        from concourse.bass_utils import run_bass_kernel_spmd

        if E_pad not in _compiled_cache:
            _compiled_cache[E_pad] = _build_graph(E_pad)
        nc = _compiled_cache[E_pad]
        trace = bool(os.environ.get("BASS_TRACE"))
        LAST_RESULTS = run_bass_kernel_spmd(
            nc, in_maps, list(range(N_CORES)), trace=trace,
        )
        outs = [r["out"] for r in LAST_RESULTS.results]

    out_sorted = np.empty(E, dtype=np.float32)
    for k in range(N_CORES):
        e0, e1 = int(core_e[k]), int(core_e[k + 1])
        out_sorted[e0:e1] = np.asarray(outs[k]).astype(np.float32).ravel()[reasm[k]]
    out = np.empty(E, dtype=np.float32)
    out[order] = out_sorted
    return out[:, None]
